# revision 12
# baseline (speedup 1.0000x reference)
"""BiRNN (Bowman SNLI) Trainium2 kernel.

Sharding: 8 cores = 4 LSTM directions x 2 batch halves (SPMD — same program,
per-core weights/inputs differ). Each core runs one LSTM (batch 128, T=128):
per step z = [x_t, h, 1] @ W_aug accumulated in PSUM with stationary =
x_t^T / h^T chunks and moving = weight columns (N=512, bf16). Gates are
column-reordered [i, f, o, j] host-side and the forget bias folded into b.

Per-step emission order keeps the PE busy: x-part of step t, then the four
h^T transposes of step t-1 interleaved with step t's h-part matmuls so the
recurrence never stalls the tensor engine. The elementwise tail is spread
over Vector (i*j, c, h), GpSimd (c*f), and Scalar (gates/tanh + hT copies).

The MLP is K-sharded: each core computes its direction's slice of layer-1
pre-activations, a 4-core AllReduce sums them, and every core then runs
layers 2-4 redundantly in wide-N layout (PE transposes between layers).
Cores 0 and 4 write logits^T [3, 128] which the host reads.
"""
import numpy as np
import ml_dtypes

# Harness-visible constants
B, T, E, H, F = 256, 128, 300, 512, 1024
BC = 128          # batch per core
N_CORES = 8
EP = 384          # padded x feature dim (300 x + 1 bias + pad)
KX = 3            # x stationary chunks (last has 45 valid rows)
KH = 4            # h stationary chunks

_cache = {}


def _apply_tile_patch():
    """walrus here allows ONE semaphore wait per instruction; Tile's tail
    drain (and occasionally other instructions) get more. Split extra waits
    onto same-engine NoOp carriers inserted immediately before."""
    import concourse.tile as tile
    import concourse.mybir as mybir
    from concourse.tile import ScopedClock

    if getattr(tile.TileContext, "_multiwait_patched", False):
        return

    def split_multiwait(nc):
        for f in nc.m.functions:
            for bb in f.blocks:
                insts = bb.instructions
                if not any(
                    i.sync_info is not None and len(i.sync_info.on_wait) > 1
                    for i in insts
                ):
                    continue
                new = []
                for inst in insts:
                    si = inst.sync_info
                    if si is not None and len(si.on_wait) > 1:
                        waits = list(si.on_wait)
                        for w in waits[:-1]:
                            carrier = mybir.InstNoOp(
                                name=nc.get_next_instruction_name(), ins=[], outs=[]
                            )
                            carrier.engine = inst.engine
                            carrier.sync_info = mybir.SyncInfo(
                                on_wait=[w], on_update=[]
                            )
                            nc.register_instruction(carrier, overwrite=True)
                            new.append(carrier)
                        si.on_wait = [waits[-1]]
                    new.append(inst)
                bb.instructions = new

    def _patched_drain_and_barrier(self, tick_clock, wait_clock):
        nc = self.nc
        drain_inst = nc.sync.drain()
        wait_clock.add_sem_waits(
            drain_inst.ins, ScopedClock({None: tick_clock.global_clock})
        )
        nc.all_engine_barrier()
        assert self.sems is not None
        popped = nc._tile_sem_poison_stack.pop()
        assert popped is self._sem_poison
        nc.clear_and_free_semaphores(list(self.sems.allocated().values()))
        nc.all_engine_barrier()
        split_multiwait(nc)

    tile.TileContext._drain_and_barrier = _patched_drain_and_barrier
    tile.TileContext._multiwait_patched = True


def _build_nc(t_steps=T, cc_mode="ar4"):
    _apply_tile_patch()
    from contextlib import ExitStack
    import concourse.bass as bass
    import concourse.tile as tile
    from concourse import mybir

    f32 = mybir.dt.float32
    bf16 = mybir.dt.bfloat16
    AF = mybir.ActivationFunctionType

    nc = bass.Bass("TRN2", target_bir_lowering=False, debug=False,
                   num_devices=N_CORES)

    G4 = 4 * H  # 2048

    xt_d = nc.dram_tensor("xt", [t_steps, 128, KX * 128], bf16, kind="ExternalInput").ap()
    wl_d = nc.dram_tensor("wl", [128, KX + KH, G4], bf16, kind="ExternalInput").ap()
    w1_d = nc.dram_tensor("w1", [128, 4, F], bf16, kind="ExternalInput").ap()
    w2_d = nc.dram_tensor("w2", [128, 8, F], bf16, kind="ExternalInput").ap()
    w3_d = nc.dram_tensor("w3", [128, 8, F], bf16, kind="ExternalInput").ap()
    w4_d = nc.dram_tensor("w4", [128, 8, 3], bf16, kind="ExternalInput").ap()
    b1_d = nc.dram_tensor("b1", [1, F], bf16, kind="ExternalInput").ap()
    b2_d = nc.dram_tensor("b2", [1, F], bf16, kind="ExternalInput").ap()
    b3_d = nc.dram_tensor("b3", [1, F], bf16, kind="ExternalInput").ap()
    b4_d = nc.dram_tensor("b4", [1, 3], bf16, kind="ExternalInput").ap()
    ones_d = nc.dram_tensor("ones", [1, 128], bf16, kind="ExternalInput").ap()
    idr_d = nc.dram_tensor("identr", [128, 128], bf16, kind="ExternalInput").ap()
    out_d = nc.dram_tensor("logitsT", [3, 128], f32, kind="ExternalOutput").ap()

    crin = nc.dram_tensor("crin", [128, F], bf16)
    crout = nc.dram_tensor("crout", [128, F], bf16)

    with tile.TileContext(nc) as tc, ExitStack() as ctx:
        wp = ctx.enter_context(tc.tile_pool(name="weights", bufs=1))
        wl_sb = wp.tile([128, KX + KH, G4], bf16, tag="wl")
        w1_sb = wp.tile([128, 4, F], bf16, tag="w1")
        w2_sb = wp.tile([128, 8, F], bf16, tag="w2")
        w3_sb = wp.tile([128, 8, F], bf16, tag="w3")
        w4_sb = wp.tile([128, 8, 3], bf16, tag="w4")
        b1_sb = wp.tile([1, F], bf16, tag="b1")
        b2_sb = wp.tile([1, F], bf16, tag="b2")
        b3_sb = wp.tile([1, F], bf16, tag="b3")
        b4_sb = wp.tile([1, 3], bf16, tag="b4")
        ones_sb = wp.tile([1, 128], bf16, tag="ones")
        idr_sb = wp.tile([128, 128], bf16, tag="idr")
        # LSTM-critical loads first, split per (chunk, bank) in the order the
        # x-part consumes them so the first matmul starts after ~128KB of
        # weight DMA instead of the full 3.5MB
        xp = ctx.enter_context(tc.tile_pool(name="xsteps", bufs=4))
        xt0_sb = xp.tile([128, KX * 128], bf16, tag="xt", name="xt0")
        for n in (0, 3, 1, 2):
            ns = slice(n * 512, (n + 1) * 512)
            for k in range(KX):
                nc.sync.dma_start(wl_sb[:, k, ns], wl_d[:, k, ns])
            if n == 0:
                nc.sync.dma_start(xt0_sb[:], xt_d[0])
                nc.sync.dma_start(idr_sb[:], idr_d[:])
        for n in (0, 3, 1, 2):
            ns = slice(n * 512, (n + 1) * 512)
            for k in range(KX, KX + KH):
                nc.sync.dma_start(wl_sb[:, k, ns], wl_d[:, k, ns])
        # MLP weights aren't needed until after the T loop — emitted late
        # (below) so their DMA doesn't delay the LSTM start.
        _mlp_loads = [(w1_sb, w1_d), (w2_sb, w2_d), (w3_sb, w3_d),
                      (w4_sb, w4_d), (b1_sb, b1_d), (b2_sb, b2_d),
                      (b3_sb, b3_d), (b4_sb, b4_d), (ones_sb, ones_d)]

        sp = ctx.enter_context(tc.tile_pool(name="state", bufs=2))

        c_prev = None
        h_pend = None     # h of step t-1, awaiting transpose (2 half tiles)
        hT_chunks = None  # hT chunk tiles for this step's h-part
        cT_bf = None

        # gate layout [i | f | o | j]; bank order = ACT completion order
        BANK_ORDER = (0, 3, 1, 2)
        GATE_FUNC = {0: AF.Sigmoid, 1: AF.Sigmoid, 2: AF.Sigmoid, 3: AF.Tanh}

        with tc.tile_pool(name="zpsum", bufs=1, space="PSUM") as zpool, \
             tc.tile_pool(name="trpsum", bufs=4, space="PSUM") as trpool:
            for t in range(t_steps):
                if t == 2:
                    for sb_t, d in _mlp_loads:
                        nc.sync.dma_start(sb_t[:], d[:])
                if t == 0:
                    xt_sb = xt0_sb
                else:
                    xt_sb = xp.tile([128, KX * 128], bf16, tag="xt")
                    nc.sync.dma_start(xt_sb[:], xt_d[t])
                zb = [zpool.tile([128, 512], f32, tag=f"z{n}", name=f"zb{n}")
                      for n in range(4)]

                # x-part: bank-major in ACT completion order so the WAR wait
                # on last step's gate read is already satisfied
                for n in BANK_ORDER:
                    ns = slice(n * 512, (n + 1) * 512)
                    for k in range(KX):
                        nc.tensor.matmul(
                            zb[n][:],
                            xt_sb[:, k * 128:(k + 1) * 128],
                            wl_sb[:, k, ns],
                            start=(k == 0),
                            stop=(t == 0 and k == KX - 1),
                        )

                if h_pend is not None:
                    # transpose h_{t-1} chunk k, copy to SBUF, then step t's
                    # chunk-k h-matmuls — PE alternates transpose/matmul with
                    # no idle, consuming h halves as the tail produces them
                    hT_chunks = [
                        sp.tile([128, 128], bf16, tag=f"hT{k}", name=f"hT{k}")
                        for k in range(KH)
                    ]
                    for k in range(KH):
                        trp = trpool.tile([128, 128], bf16, tag="tr")
                        nc.tensor.transpose(
                            trp[:], h_pend[k // 2][:, (k % 2) * 128:(k % 2) * 128 + 128],
                            idr_sb[:],
                        )
                        if k % 2 == 0:
                            nc.scalar.copy(hT_chunks[k][:], trp[:])
                        else:
                            nc.vector.tensor_copy(hT_chunks[k][:], trp[:])
                        if k < 2:
                            for n in BANK_ORDER:
                                ns = slice(n * 512, (n + 1) * 512)
                                nc.tensor.matmul(
                                    zb[n][:], hT_chunks[k][:], wl_sb[:, KX + k, ns],
                                    start=False, stop=False,
                                )
                    # finish banks in ACT order for chunks 2,3
                    for n in BANK_ORDER:
                        ns = slice(n * 512, (n + 1) * 512)
                        for k in (2, 3):
                            nc.tensor.matmul(
                                zb[n][:], hT_chunks[k][:], wl_sb[:, KX + k, ns],
                                start=False, stop=(k == KH - 1),
                            )

                gates = sp.tile([128, G4], f32, tag="gates")
                for n in BANK_ORDER:
                    nc.scalar.activation(
                        gates[:, n * 512:(n + 1) * 512], zb[n][:], GATE_FUNC[n]
                    )

                # elementwise tail, two H-halves; c*f on gpsimd to keep the
                # Vector queue short (i*j, c add, h mul)
                t2 = sp.tile([128, H], f32, tag="t2")
                if t == 0:
                    c_new = t2
                else:
                    t1 = sp.tile([128, H], f32, tag="t1")
                    c_new = sp.tile([128, H], f32, tag="c")
                last = t == t_steps - 1
                if not last:
                    tanc = sp.tile([128, H], f32, tag="tanc")
                    h_new = [sp.tile([128, 256], bf16, tag=f"h{hf}",
                                     name=f"h{hf}")
                             for hf in (0, 1)]
                for hf in (0, 1):
                    sl = slice(hf * 256, (hf + 1) * 256)
                    gi = gates[:, hf * 256:hf * 256 + 256]
                    gf = gates[:, 512 + hf * 256:512 + hf * 256 + 256]
                    go = gates[:, 1024 + hf * 256:1024 + hf * 256 + 256]
                    gj = gates[:, 1536 + hf * 256:1536 + hf * 256 + 256]
                    nc.vector.tensor_mul(t2[:, sl], gi, gj)
                    if t > 0:
                        nc.gpsimd.tensor_mul(t1[:, sl], c_prev[:, sl], gf)
                        nc.vector.tensor_add(c_new[:, sl], t1[:, sl], t2[:, sl])
                    if not last:
                        nc.scalar.activation(tanc[:, sl], c_new[:, sl], AF.Tanh)
                        nc.vector.tensor_mul(h_new[hf][:], tanc[:, sl], go)
                c_prev = c_new
                if not last:
                    h_pend = h_new

            # final cell state -> bf16, transposed into cT [p, k*128+b]
            cb = sp.tile([128, H], bf16, tag="cb")
            nc.vector.tensor_copy(cb[:], c_prev[:])
            cT_bf = sp.tile([128, H], bf16, tag="cT")
            for k in range(4):
                ks = slice(k * 128, (k + 1) * 128)
                trp = trpool.tile([128, 128], bf16, tag="tr")
                nc.tensor.transpose(trp[:], cb[:, ks], idr_sb[:])
                nc.vector.tensor_copy(cT_bf[:, ks], trp[:])

        # layer-1 partial for this core's direction slice of W1, then
        # AllReduce the partial pre-activations within each batch-half group
        with tc.tile_pool(name="mlppsum", bufs=1, space="PSUM") as mp, \
             tc.tile_pool(name="mtrpsum", bufs=3, space="PSUM") as mtr:
            aps = mp.tile([128, F], f32, tag="aps1")
            for half in (0, 1):
                ms = slice(half * 512, (half + 1) * 512)
                for kc in range(4):
                    nc.tensor.matmul(
                        aps[:, ms], cT_bf[:, kc * 128:(kc + 1) * 128],
                        w1_sb[:, kc, ms], start=(kc == 0), stop=False,
                    )
                nc.tensor.matmul(aps[:, ms], ones_sb[0:1, :], b1_sb[0:1, ms],
                                 start=False, stop=True)
            part1 = sp.tile([128, F], bf16, tag="part1")
            nc.vector.tensor_copy(part1[:], aps[:])
            nc.sync.dma_start(crin.ap()[:], part1[:])
            nc.gpsimd.collective_compute(
                "AllReduce",
                mybir.AluOpType.add,
                replica_groups=[[0, 1, 2, 3], [4, 5, 6, 7]],
                ins=[crin.ap()[:]],
                outs=[crout.ap()[:]],
            )
            s1 = sp.tile([128, F], bf16, tag="s1")
            nc.sync.dma_start(s1[:], crout.ap()[:])
            act_in = sp.tile([128, F], bf16, tag="a1")
            nc.scalar.activation(act_in[:], s1[:], AF.Tanh)

            # layers 2..3 wide-N: transpose a, then psum = aT @ W + b, tanh
            for li, (w_sb, b_sb) in enumerate([(w2_sb, b2_sb), (w3_sb, b3_sb)]):
                aT = sp.tile([128, F], bf16, tag=f"aT{li}")
                for m in range(8):
                    ms = slice(m * 128, (m + 1) * 128)
                    trp = mtr.tile([128, 128], bf16, tag="mtr")
                    nc.tensor.transpose(trp[:], act_in[:, ms], idr_sb[:])
                    if m % 2 == 0:
                        nc.scalar.copy(aT[:, ms], trp[:])
                    else:
                        nc.vector.tensor_copy(aT[:, ms], trp[:])
                aps2 = mp.tile([128, F], f32, tag="apsL", name="aps2")
                for half in (0, 1):
                    ms = slice(half * 512, (half + 1) * 512)
                    for kc in range(8):
                        nc.tensor.matmul(
                            aps2[:, ms], aT[:, kc * 128:(kc + 1) * 128],
                            w_sb[:, kc, ms], start=(kc == 0), stop=False,
                        )
                    nc.tensor.matmul(aps2[:, ms], ones_sb[0:1, :], b_sb[0:1, ms],
                                     start=False, stop=True)
                nxt = sp.tile([128, F], bf16, tag=f"a{li + 2}")
                nc.scalar.activation(nxt[:], aps2[:], AF.Tanh)
                act_in = nxt

            # layer 4 in [3, batch] layout: stationary = W4 chunks
            a3T = sp.tile([128, F], bf16, tag="a3T")
            for m in range(8):
                ms = slice(m * 128, (m + 1) * 128)
                trp = mtr.tile([128, 128], bf16, tag="mtr")
                nc.tensor.transpose(trp[:], act_in[:, ms], idr_sb[:])
                if m % 2 == 0:
                    nc.scalar.copy(a3T[:, ms], trp[:])
                else:
                    nc.vector.tensor_copy(a3T[:, ms], trp[:])
            l4 = mp.tile([3, 128], f32, tag="l4")
            for kc in range(8):
                nc.tensor.matmul(
                    l4[:], w4_sb[:, kc, :], a3T[:, kc * 128:(kc + 1) * 128],
                    start=(kc == 0), stop=False,
                )
            nc.tensor.matmul(l4[:], b4_sb[0:1, :], ones_sb[0:1, :],
                             start=False, stop=True)
            lg = sp.tile([3, 128], f32, tag="lg")
            nc.scalar.copy(lg[:], l4[:])
            nc.sync.dma_start(out_d[:], lg[:])

    return nc


def _pack_core_inputs(core, inputs, t_steps=T):
    """Build the per-core in_map (numpy only)."""
    bf16 = ml_dtypes.bfloat16
    lstm = core % 4
    half = core // 4
    rows = slice(half * BC, (half + 1) * BC)

    if lstm < 2:
        x = np.asarray(inputs["premises"])[rows]
        W = np.asarray(inputs["W_fw_p"] if lstm == 0 else inputs["W_bw_p"])
        b = np.asarray(inputs["b_fw_p"] if lstm == 0 else inputs["b_bw_p"])
    else:
        x = np.asarray(inputs["hypotheses"])[rows]
        W = np.asarray(inputs["W_fw_h"] if lstm == 2 else inputs["W_bw_h"])
        b = np.asarray(inputs["b_fw_h"] if lstm == 2 else inputs["b_bw_h"])
    x = x[:, :t_steps]
    if lstm % 2 == 1:
        x = x[:, ::-1, :]

    # gate reorder [i, f, o, j]; fold forget_bias=1.0 into b
    perm = np.concatenate([
        np.arange(0, H), np.arange(2 * H, 3 * H),
        np.arange(3 * H, 4 * H), np.arange(H, 2 * H),
    ])
    Wp = W[:, perm].astype(np.float32)
    bp = b[perm].astype(np.float32).copy()
    bp[H:2 * H] += 1.0  # forget gate slice in new layout

    xa = np.zeros((BC, t_steps, EP), np.float32)
    xa[:, :, :E] = x
    xa[:, :, E] = 1.0
    xt = np.ascontiguousarray(
        xa.reshape(BC, t_steps, KX, 128).transpose(1, 3, 2, 0)
    ).reshape(t_steps, 128, KX * 128)

    wl = np.zeros((128, KX + KH, 4 * H), np.float32)
    W_aug_x = np.zeros((EP, 4 * H), np.float32)
    W_aug_x[:E] = Wp[:E]
    W_aug_x[E] = bp
    for k in range(KX):
        wl[:, k, :] = W_aug_x[k * 128:(k + 1) * 128]
    for k in range(KH):
        wl[:, KX + k, :] = Wp[E + k * 128: E + (k + 1) * 128]

    W1 = np.asarray(inputs["W1"]).astype(np.float32)
    W2 = np.asarray(inputs["W2"]).astype(np.float32)
    W3 = np.asarray(inputs["W3"]).astype(np.float32)
    W4 = np.asarray(inputs["W4"]).astype(np.float32)
    # per-core W1 slice: rows for this core's direction in the rnn concat
    w1s = W1[512 * lstm:512 * (lstm + 1)].reshape(4, 128, F).transpose(1, 0, 2)
    w2 = W2.reshape(8, 128, F).transpose(1, 0, 2).astype(bf16)
    w3 = W3.reshape(8, 128, F).transpose(1, 0, 2).astype(bf16)
    w4 = W4.reshape(8, 128, 3).transpose(1, 0, 2).astype(bf16)

    return {
        "xt": xt.astype(bf16),
        "wl": wl.astype(bf16),
        "w1": np.ascontiguousarray(w1s).astype(bf16),
        "w2": np.ascontiguousarray(w2),
        "w3": np.ascontiguousarray(w3),
        "w4": np.ascontiguousarray(w4),
        # b1 scaled by 1/4: each of the 4 group cores adds it once into the
        # AllReduce sum
        "b1": (np.asarray(inputs["b1"]).reshape(1, F) * 0.25).astype(bf16),
        "b2": np.asarray(inputs["b2"]).reshape(1, F).astype(bf16),
        "b3": np.asarray(inputs["b3"]).reshape(1, F).astype(bf16),
        "b4": np.asarray(inputs["b4"]).reshape(1, 3).astype(bf16),
        "ones": np.ones((1, 128), bf16),
        "identr": np.eye(128, dtype=bf16),
    }


def _install_ntff_shim():
    """This image's `antenv` lacks `axon_hooks`; provide it so
    run_bass_kernel_spmd(trace=True) can capture NTFF profiles."""
    import sys
    import types

    if "antenv.axon_hooks" in sys.modules:
        return
    mod = types.ModuleType("antenv.axon_hooks")
    state = {"hook": None}
    mod.set_axon_ntff_profile_hook = lambda h: state.__setitem__("hook", h)
    mod.get_axon_ntff_profile_hook = lambda: state["hook"]
    sys.modules["antenv.axon_hooks"] = mod
    try:
        from trn_agent_boot.trn_boot import _ntff_profile_via_ctypes

        mod.set_axon_ntff_profile_hook(
            _ntff_profile_via_ctypes("/opt/axon/libaxon_pjrt.so")
        )
    except Exception:
        pass


def _run(inputs, trace=False, t_steps=T, ldt="bf16"):
    if trace:
        _install_ntff_shim()
    from concourse.bass_utils import run_bass_kernel_spmd

    key = (t_steps,)
    if key not in _cache:
        _cache[key] = _build_nc(t_steps)
    nc = _cache[key]
    in_maps = [_pack_core_inputs(c, inputs, t_steps) for c in range(N_CORES)]
    res = run_bass_kernel_spmd(
        nc, in_maps, list(range(N_CORES)), trace=trace
    )
    out = np.zeros((B, 3), np.float32)
    out[0:BC] = res.results[0]["logitsT"].T
    out[BC:2 * BC] = res.results[4]["logitsT"].T
    return out, res


def kernel(**inputs) -> np.ndarray:
    out, _ = _run(inputs, trace=False)
    return out


# revision 13
# speedup vs baseline: 1.0108x; 1.0108x over previous
"""BiRNN (Bowman SNLI) Trainium2 kernel.

Sharding: 8 cores = 4 LSTM directions x 2 batch halves (SPMD — same program,
per-core weights/inputs differ). Each core runs one LSTM (batch 128, T=128):
per step z = [x_t, h, 1] @ W_aug accumulated in PSUM with stationary =
x_t^T / h^T chunks and moving = weight columns (N=512, bf16). Gates are
column-reordered [i, f, o, j] host-side and the forget bias folded into b.

Per-step emission order keeps the PE busy: x-part of step t, then the four
h^T transposes of step t-1 interleaved with step t's h-part matmuls so the
recurrence never stalls the tensor engine. The elementwise tail is spread
over Vector (i*j, c, h), GpSimd (c*f), and Scalar (gates/tanh + hT copies).

The MLP is K-sharded: each core computes its direction's slice of layer-1
pre-activations, a 4-core AllReduce sums them, and every core then runs
layers 2-4 redundantly in wide-N layout (PE transposes between layers).
Cores 0 and 4 write logits^T [3, 128] which the host reads.
"""
import numpy as np
import ml_dtypes

# Harness-visible constants
B, T, E, H, F = 256, 128, 300, 512, 1024
BC = 128          # batch per core
N_CORES = 8
EP = 384          # padded x feature dim (300 x + 1 bias + pad)
KX = 3            # x stationary chunks (last has 45 valid rows)
KH = 4            # h stationary chunks

_cache = {}


def _apply_tile_patch():
    """walrus here allows ONE semaphore wait per instruction; Tile's tail
    drain (and occasionally other instructions) get more. Split extra waits
    onto same-engine NoOp carriers inserted immediately before."""
    import concourse.tile as tile
    import concourse.mybir as mybir
    from concourse.tile import ScopedClock

    if getattr(tile.TileContext, "_multiwait_patched", False):
        return

    def split_multiwait(nc):
        for f in nc.m.functions:
            for bb in f.blocks:
                insts = bb.instructions
                if not any(
                    i.sync_info is not None and len(i.sync_info.on_wait) > 1
                    for i in insts
                ):
                    continue
                new = []
                for inst in insts:
                    si = inst.sync_info
                    if si is not None and len(si.on_wait) > 1:
                        waits = list(si.on_wait)
                        for w in waits[:-1]:
                            carrier = mybir.InstNoOp(
                                name=nc.get_next_instruction_name(), ins=[], outs=[]
                            )
                            carrier.engine = inst.engine
                            carrier.sync_info = mybir.SyncInfo(
                                on_wait=[w], on_update=[]
                            )
                            nc.register_instruction(carrier, overwrite=True)
                            new.append(carrier)
                        si.on_wait = [waits[-1]]
                    new.append(inst)
                bb.instructions = new

    def _patched_drain_and_barrier(self, tick_clock, wait_clock):
        nc = self.nc
        drain_inst = nc.sync.drain()
        wait_clock.add_sem_waits(
            drain_inst.ins, ScopedClock({None: tick_clock.global_clock})
        )
        nc.all_engine_barrier()
        assert self.sems is not None
        popped = nc._tile_sem_poison_stack.pop()
        assert popped is self._sem_poison
        nc.clear_and_free_semaphores(list(self.sems.allocated().values()))
        nc.all_engine_barrier()
        split_multiwait(nc)

    tile.TileContext._drain_and_barrier = _patched_drain_and_barrier
    tile.TileContext._multiwait_patched = True


def _build_nc(t_steps=T, cc_mode="ar4"):
    _apply_tile_patch()
    from contextlib import ExitStack
    import concourse.bass as bass
    import concourse.tile as tile
    from concourse import mybir

    f32 = mybir.dt.float32
    bf16 = mybir.dt.bfloat16
    AF = mybir.ActivationFunctionType

    nc = bass.Bass("TRN2", target_bir_lowering=False, debug=False,
                   num_devices=N_CORES)

    G4 = 4 * H  # 2048

    xt_d = nc.dram_tensor("xt", [t_steps, 128, KX * 128], bf16, kind="ExternalInput").ap()
    wl_d = nc.dram_tensor("wl", [128, KX + KH, G4], bf16, kind="ExternalInput").ap()
    w1_d = nc.dram_tensor("w1", [128, 4, F], bf16, kind="ExternalInput").ap()
    w2_d = nc.dram_tensor("w2", [128, 8, F], bf16, kind="ExternalInput").ap()
    w3_d = nc.dram_tensor("w3", [128, 8, F], bf16, kind="ExternalInput").ap()
    w4_d = nc.dram_tensor("w4", [128, 8, 3], bf16, kind="ExternalInput").ap()
    b1_d = nc.dram_tensor("b1", [1, F], bf16, kind="ExternalInput").ap()
    b2_d = nc.dram_tensor("b2", [1, F], bf16, kind="ExternalInput").ap()
    b3_d = nc.dram_tensor("b3", [1, F], bf16, kind="ExternalInput").ap()
    b4_d = nc.dram_tensor("b4", [1, 3], bf16, kind="ExternalInput").ap()
    ones_d = nc.dram_tensor("ones", [1, 128], bf16, kind="ExternalInput").ap()
    idr_d = nc.dram_tensor("identr", [128, 128], bf16, kind="ExternalInput").ap()
    out_d = nc.dram_tensor("logitsT", [3, 128], f32, kind="ExternalOutput").ap()

    crin = nc.dram_tensor("crin", [128, F], bf16)
    crout = nc.dram_tensor("crout", [128, F], bf16)

    with tile.TileContext(nc) as tc, ExitStack() as ctx:
        wp = ctx.enter_context(tc.tile_pool(name="weights", bufs=1))
        wl_sb = wp.tile([128, KX + KH, G4], bf16, tag="wl")
        w1_sb = wp.tile([128, 4, F], bf16, tag="w1")
        w2_sb = wp.tile([128, 8, F], bf16, tag="w2")
        w3_sb = wp.tile([128, 8, F], bf16, tag="w3")
        w4_sb = wp.tile([128, 8, 3], bf16, tag="w4")
        b1_sb = wp.tile([1, F], bf16, tag="b1")
        b2_sb = wp.tile([1, F], bf16, tag="b2")
        b3_sb = wp.tile([1, F], bf16, tag="b3")
        b4_sb = wp.tile([1, 3], bf16, tag="b4")
        ones_sb = wp.tile([1, 128], bf16, tag="ones")
        idr_sb = wp.tile([128, 128], bf16, tag="idr")
        # LSTM-critical loads first, split per (chunk, bank) in the order the
        # x-part consumes them so the first matmul starts after ~128KB of
        # weight DMA instead of the full 3.5MB
        xp = ctx.enter_context(tc.tile_pool(name="xsteps", bufs=4))
        xt0_sb = xp.tile([128, KX * 128], bf16, tag="xt", name="xt0")
        nc.sync.dma_start(wl_sb[:, 0, :], wl_d[:, 0])
        nc.sync.dma_start(xt0_sb[:], xt_d[0])
        for k in range(1, KX):
            nc.sync.dma_start(wl_sb[:, k, :], wl_d[:, k])
        nc.sync.dma_start(idr_sb[:], idr_d[:])
        for k in range(KX, KX + KH):
            nc.sync.dma_start(wl_sb[:, k, :], wl_d[:, k])
        # MLP weights aren't needed until after the T loop — emitted late
        # (below) so their DMA doesn't delay the LSTM start.
        _mlp_loads = [(w1_sb, w1_d), (w2_sb, w2_d), (w3_sb, w3_d),
                      (w4_sb, w4_d), (b1_sb, b1_d), (b2_sb, b2_d),
                      (b3_sb, b3_d), (b4_sb, b4_d), (ones_sb, ones_d)]

        sp = ctx.enter_context(tc.tile_pool(name="state", bufs=2))

        c_prev = None
        h_pend = None     # h of step t-1, awaiting transpose (2 half tiles)
        hT_chunks = None  # hT chunk tiles for this step's h-part
        cT_bf = None

        # gate layout [i | f | o | j]; bank order = ACT completion order
        BANK_ORDER = (0, 3, 1, 2)
        GATE_FUNC = {0: AF.Sigmoid, 1: AF.Sigmoid, 2: AF.Sigmoid, 3: AF.Tanh}

        with tc.tile_pool(name="zpsum", bufs=1, space="PSUM") as zpool, \
             tc.tile_pool(name="trpsum", bufs=4, space="PSUM") as trpool:
            for t in range(t_steps):
                if t == 2:
                    for sb_t, d in _mlp_loads:
                        nc.sync.dma_start(sb_t[:], d[:])
                if t == 0:
                    xt_sb = xt0_sb
                else:
                    xt_sb = xp.tile([128, KX * 128], bf16, tag="xt")
                    nc.sync.dma_start(xt_sb[:], xt_d[t])
                zb = [zpool.tile([128, 512], f32, tag=f"z{n}", name=f"zb{n}")
                      for n in range(4)]

                # x-part: bank-major in ACT completion order so the WAR wait
                # on last step's gate read is already satisfied
                for n in BANK_ORDER:
                    ns = slice(n * 512, (n + 1) * 512)
                    for k in range(KX):
                        nc.tensor.matmul(
                            zb[n][:],
                            xt_sb[:, k * 128:(k + 1) * 128],
                            wl_sb[:, k, ns],
                            start=(k == 0),
                            stop=(t == 0 and k == KX - 1),
                        )

                if h_pend is not None:
                    # transpose h_{t-1} chunk k, copy to SBUF, then step t's
                    # chunk-k h-matmuls — PE alternates transpose/matmul with
                    # no idle, consuming h halves as the tail produces them
                    hT_chunks = [
                        sp.tile([128, 128], bf16, tag=f"hT{k}", name=f"hT{k}")
                        for k in range(KH)
                    ]
                    for k in range(KH):
                        trp = trpool.tile([128, 128], bf16, tag="tr")
                        nc.tensor.transpose(
                            trp[:], h_pend[k // 2][:, (k % 2) * 128:(k % 2) * 128 + 128],
                            idr_sb[:],
                        )
                        if k % 2 == 0:
                            nc.scalar.copy(hT_chunks[k][:], trp[:])
                        else:
                            nc.vector.tensor_copy(hT_chunks[k][:], trp[:])
                        if k < 2:
                            for n in BANK_ORDER:
                                ns = slice(n * 512, (n + 1) * 512)
                                nc.tensor.matmul(
                                    zb[n][:], hT_chunks[k][:], wl_sb[:, KX + k, ns],
                                    start=False, stop=False,
                                )
                    # finish banks in ACT order for chunks 2,3
                    for n in BANK_ORDER:
                        ns = slice(n * 512, (n + 1) * 512)
                        for k in (2, 3):
                            nc.tensor.matmul(
                                zb[n][:], hT_chunks[k][:], wl_sb[:, KX + k, ns],
                                start=False, stop=(k == KH - 1),
                            )

                gates = sp.tile([128, G4], f32, tag="gates")
                for n in BANK_ORDER:
                    nc.scalar.activation(
                        gates[:, n * 512:(n + 1) * 512], zb[n][:], GATE_FUNC[n]
                    )

                # elementwise tail, two H-halves; c*f on gpsimd to keep the
                # Vector queue short (i*j, c add, h mul)
                t2 = sp.tile([128, H], f32, tag="t2")
                if t == 0:
                    c_new = t2
                else:
                    t1 = sp.tile([128, H], f32, tag="t1")
                    c_new = sp.tile([128, H], f32, tag="c")
                last = t == t_steps - 1
                if not last:
                    tanc = sp.tile([128, H], f32, tag="tanc")
                    h_new = [sp.tile([128, 256], bf16, tag=f"h{hf}",
                                     name=f"h{hf}")
                             for hf in (0, 1)]
                for hf in (0, 1):
                    sl = slice(hf * 256, (hf + 1) * 256)
                    gi = gates[:, hf * 256:hf * 256 + 256]
                    gf = gates[:, 512 + hf * 256:512 + hf * 256 + 256]
                    go = gates[:, 1024 + hf * 256:1024 + hf * 256 + 256]
                    gj = gates[:, 1536 + hf * 256:1536 + hf * 256 + 256]
                    nc.vector.tensor_mul(t2[:, sl], gi, gj)
                    if t > 0:
                        nc.gpsimd.tensor_mul(t1[:, sl], c_prev[:, sl], gf)
                        nc.vector.tensor_add(c_new[:, sl], t1[:, sl], t2[:, sl])
                    if not last:
                        nc.scalar.activation(tanc[:, sl], c_new[:, sl], AF.Tanh)
                        nc.vector.tensor_mul(h_new[hf][:], tanc[:, sl], go)
                c_prev = c_new
                if not last:
                    h_pend = h_new

            # final cell state -> bf16, transposed into cT [p, k*128+b]
            cb = sp.tile([128, H], bf16, tag="cb")
            nc.vector.tensor_copy(cb[:], c_prev[:])
            cT_bf = sp.tile([128, H], bf16, tag="cT")
            for k in range(4):
                ks = slice(k * 128, (k + 1) * 128)
                trp = trpool.tile([128, 128], bf16, tag="tr")
                nc.tensor.transpose(trp[:], cb[:, ks], idr_sb[:])
                nc.vector.tensor_copy(cT_bf[:, ks], trp[:])

        # layer-1 partial for this core's direction slice of W1, then
        # AllReduce the partial pre-activations within each batch-half group
        with tc.tile_pool(name="mlppsum", bufs=1, space="PSUM") as mp, \
             tc.tile_pool(name="mtrpsum", bufs=3, space="PSUM") as mtr:
            aps = mp.tile([128, F], f32, tag="aps1")
            for half in (0, 1):
                ms = slice(half * 512, (half + 1) * 512)
                for kc in range(4):
                    nc.tensor.matmul(
                        aps[:, ms], cT_bf[:, kc * 128:(kc + 1) * 128],
                        w1_sb[:, kc, ms], start=(kc == 0), stop=False,
                    )
                nc.tensor.matmul(aps[:, ms], ones_sb[0:1, :], b1_sb[0:1, ms],
                                 start=False, stop=True)
            part1 = sp.tile([128, F], bf16, tag="part1")
            nc.vector.tensor_copy(part1[:], aps[:])
            nc.sync.dma_start(crin.ap()[:], part1[:])
            nc.gpsimd.collective_compute(
                "AllReduce",
                mybir.AluOpType.add,
                replica_groups=[[0, 1, 2, 3], [4, 5, 6, 7]],
                ins=[crin.ap()[:]],
                outs=[crout.ap()[:]],
            )
            s1 = sp.tile([128, F], bf16, tag="s1")
            nc.sync.dma_start(s1[:], crout.ap()[:])
            act_in = sp.tile([128, F], bf16, tag="a1")
            nc.scalar.activation(act_in[:], s1[:], AF.Tanh)

            # layers 2..3 wide-N: transpose a, then psum = aT @ W + b, tanh
            for li, (w_sb, b_sb) in enumerate([(w2_sb, b2_sb), (w3_sb, b3_sb)]):
                aT = sp.tile([128, F], bf16, tag=f"aT{li}")
                for m in range(8):
                    ms = slice(m * 128, (m + 1) * 128)
                    trp = mtr.tile([128, 128], bf16, tag="mtr")
                    nc.tensor.transpose(trp[:], act_in[:, ms], idr_sb[:])
                    if m % 2 == 0:
                        nc.scalar.copy(aT[:, ms], trp[:])
                    else:
                        nc.vector.tensor_copy(aT[:, ms], trp[:])
                aps2 = mp.tile([128, F], f32, tag="apsL", name="aps2")
                for half in (0, 1):
                    ms = slice(half * 512, (half + 1) * 512)
                    for kc in range(8):
                        nc.tensor.matmul(
                            aps2[:, ms], aT[:, kc * 128:(kc + 1) * 128],
                            w_sb[:, kc, ms], start=(kc == 0), stop=False,
                        )
                    nc.tensor.matmul(aps2[:, ms], ones_sb[0:1, :], b_sb[0:1, ms],
                                     start=False, stop=True)
                nxt = sp.tile([128, F], bf16, tag=f"a{li + 2}")
                nc.scalar.activation(nxt[:], aps2[:], AF.Tanh)
                act_in = nxt

            # layer 4 in [3, batch] layout: stationary = W4 chunks
            a3T = sp.tile([128, F], bf16, tag="a3T")
            for m in range(8):
                ms = slice(m * 128, (m + 1) * 128)
                trp = mtr.tile([128, 128], bf16, tag="mtr")
                nc.tensor.transpose(trp[:], act_in[:, ms], idr_sb[:])
                if m % 2 == 0:
                    nc.scalar.copy(a3T[:, ms], trp[:])
                else:
                    nc.vector.tensor_copy(a3T[:, ms], trp[:])
            l4 = mp.tile([3, 128], f32, tag="l4")
            for kc in range(8):
                nc.tensor.matmul(
                    l4[:], w4_sb[:, kc, :], a3T[:, kc * 128:(kc + 1) * 128],
                    start=(kc == 0), stop=False,
                )
            nc.tensor.matmul(l4[:], b4_sb[0:1, :], ones_sb[0:1, :],
                             start=False, stop=True)
            lg = sp.tile([3, 128], f32, tag="lg")
            nc.scalar.copy(lg[:], l4[:])
            nc.sync.dma_start(out_d[:], lg[:])

    return nc


def _pack_core_inputs(core, inputs, t_steps=T):
    """Build the per-core in_map (numpy only)."""
    bf16 = ml_dtypes.bfloat16
    lstm = core % 4
    half = core // 4
    rows = slice(half * BC, (half + 1) * BC)

    if lstm < 2:
        x = np.asarray(inputs["premises"])[rows]
        W = np.asarray(inputs["W_fw_p"] if lstm == 0 else inputs["W_bw_p"])
        b = np.asarray(inputs["b_fw_p"] if lstm == 0 else inputs["b_bw_p"])
    else:
        x = np.asarray(inputs["hypotheses"])[rows]
        W = np.asarray(inputs["W_fw_h"] if lstm == 2 else inputs["W_bw_h"])
        b = np.asarray(inputs["b_fw_h"] if lstm == 2 else inputs["b_bw_h"])
    x = x[:, :t_steps]
    if lstm % 2 == 1:
        x = x[:, ::-1, :]

    # gate reorder [i, f, o, j]; fold forget_bias=1.0 into b
    perm = np.concatenate([
        np.arange(0, H), np.arange(2 * H, 3 * H),
        np.arange(3 * H, 4 * H), np.arange(H, 2 * H),
    ])
    Wp = W[:, perm].astype(np.float32)
    bp = b[perm].astype(np.float32).copy()
    bp[H:2 * H] += 1.0  # forget gate slice in new layout

    xa = np.zeros((BC, t_steps, EP), np.float32)
    xa[:, :, :E] = x
    xa[:, :, E] = 1.0
    xt = np.ascontiguousarray(
        xa.reshape(BC, t_steps, KX, 128).transpose(1, 3, 2, 0)
    ).reshape(t_steps, 128, KX * 128)

    wl = np.zeros((128, KX + KH, 4 * H), np.float32)
    W_aug_x = np.zeros((EP, 4 * H), np.float32)
    W_aug_x[:E] = Wp[:E]
    W_aug_x[E] = bp
    for k in range(KX):
        wl[:, k, :] = W_aug_x[k * 128:(k + 1) * 128]
    for k in range(KH):
        wl[:, KX + k, :] = Wp[E + k * 128: E + (k + 1) * 128]

    W1 = np.asarray(inputs["W1"]).astype(np.float32)
    W2 = np.asarray(inputs["W2"]).astype(np.float32)
    W3 = np.asarray(inputs["W3"]).astype(np.float32)
    W4 = np.asarray(inputs["W4"]).astype(np.float32)
    # per-core W1 slice: rows for this core's direction in the rnn concat
    w1s = W1[512 * lstm:512 * (lstm + 1)].reshape(4, 128, F).transpose(1, 0, 2)
    w2 = W2.reshape(8, 128, F).transpose(1, 0, 2).astype(bf16)
    w3 = W3.reshape(8, 128, F).transpose(1, 0, 2).astype(bf16)
    w4 = W4.reshape(8, 128, 3).transpose(1, 0, 2).astype(bf16)

    return {
        "xt": xt.astype(bf16),
        "wl": wl.astype(bf16),
        "w1": np.ascontiguousarray(w1s).astype(bf16),
        "w2": np.ascontiguousarray(w2),
        "w3": np.ascontiguousarray(w3),
        "w4": np.ascontiguousarray(w4),
        # b1 scaled by 1/4: each of the 4 group cores adds it once into the
        # AllReduce sum
        "b1": (np.asarray(inputs["b1"]).reshape(1, F) * 0.25).astype(bf16),
        "b2": np.asarray(inputs["b2"]).reshape(1, F).astype(bf16),
        "b3": np.asarray(inputs["b3"]).reshape(1, F).astype(bf16),
        "b4": np.asarray(inputs["b4"]).reshape(1, 3).astype(bf16),
        "ones": np.ones((1, 128), bf16),
        "identr": np.eye(128, dtype=bf16),
    }


def _install_ntff_shim():
    """This image's `antenv` lacks `axon_hooks`; provide it so
    run_bass_kernel_spmd(trace=True) can capture NTFF profiles."""
    import sys
    import types

    if "antenv.axon_hooks" in sys.modules:
        return
    mod = types.ModuleType("antenv.axon_hooks")
    state = {"hook": None}
    mod.set_axon_ntff_profile_hook = lambda h: state.__setitem__("hook", h)
    mod.get_axon_ntff_profile_hook = lambda: state["hook"]
    sys.modules["antenv.axon_hooks"] = mod
    try:
        from trn_agent_boot.trn_boot import _ntff_profile_via_ctypes

        mod.set_axon_ntff_profile_hook(
            _ntff_profile_via_ctypes("/opt/axon/libaxon_pjrt.so")
        )
    except Exception:
        pass


def _run(inputs, trace=False, t_steps=T, ldt="bf16"):
    if trace:
        _install_ntff_shim()
    from concourse.bass_utils import run_bass_kernel_spmd

    key = (t_steps,)
    if key not in _cache:
        _cache[key] = _build_nc(t_steps)
    nc = _cache[key]
    in_maps = [_pack_core_inputs(c, inputs, t_steps) for c in range(N_CORES)]
    res = run_bass_kernel_spmd(
        nc, in_maps, list(range(N_CORES)), trace=trace
    )
    out = np.zeros((B, 3), np.float32)
    out[0:BC] = res.results[0]["logitsT"].T
    out[BC:2 * BC] = res.results[4]["logitsT"].T
    return out, res


def kernel(**inputs) -> np.ndarray:
    out, _ = _run(inputs, trace=False)
    return out


# revision 15
# speedup vs baseline: 1.0178x; 1.0069x over previous
"""BiRNN (Bowman SNLI) Trainium2 kernel.

Sharding: 8 cores = 4 LSTM directions x 2 batch halves (SPMD — same program,
per-core weights/inputs differ). Each core runs one LSTM (batch 128, T=128):
per step z = [x_t, h, 1] @ W_aug accumulated in PSUM with stationary =
x_t^T / h^T chunks and moving = weight columns (N=512, bf16). Gates are
column-reordered [i, f, o, j] host-side and the forget bias folded into b.

Per-step emission order keeps the PE busy: x-part of step t, then the four
h^T transposes of step t-1 interleaved with step t's h-part matmuls so the
recurrence never stalls the tensor engine. The elementwise tail is spread
over Vector (i*j, c, h), GpSimd (c*f), and Scalar (gates/tanh + hT copies).

The MLP is K-sharded: each core computes its direction's slice of layer-1
pre-activations, a 4-core AllReduce sums them, and every core then runs
layers 2-4 redundantly in wide-N layout (PE transposes between layers).
Cores 0 and 4 write logits^T [3, 128] which the host reads.
"""
import numpy as np
import ml_dtypes

# Harness-visible constants
B, T, E, H, F = 256, 128, 300, 512, 1024
BC = 128          # batch per core
N_CORES = 8
EP = 384          # padded x feature dim (300 x + 1 bias + pad)
KX = 3            # x stationary chunks (last has 45 valid rows)
KH = 4            # h stationary chunks

_cache = {}


def _apply_tile_patch():
    """walrus here allows ONE semaphore wait per instruction; Tile's tail
    drain (and occasionally other instructions) get more. Split extra waits
    onto same-engine NoOp carriers inserted immediately before."""
    import concourse.tile as tile
    import concourse.mybir as mybir
    from concourse.tile import ScopedClock

    if getattr(tile.TileContext, "_multiwait_patched", False):
        return

    def split_multiwait(nc):
        for f in nc.m.functions:
            for bb in f.blocks:
                insts = bb.instructions
                if not any(
                    i.sync_info is not None and len(i.sync_info.on_wait) > 1
                    for i in insts
                ):
                    continue
                new = []
                for inst in insts:
                    si = inst.sync_info
                    if si is not None and len(si.on_wait) > 1:
                        waits = list(si.on_wait)
                        for w in waits[:-1]:
                            carrier = mybir.InstNoOp(
                                name=nc.get_next_instruction_name(), ins=[], outs=[]
                            )
                            carrier.engine = inst.engine
                            carrier.sync_info = mybir.SyncInfo(
                                on_wait=[w], on_update=[]
                            )
                            nc.register_instruction(carrier, overwrite=True)
                            new.append(carrier)
                        si.on_wait = [waits[-1]]
                    new.append(inst)
                bb.instructions = new

    def _patched_drain_and_barrier(self, tick_clock, wait_clock):
        nc = self.nc
        drain_inst = nc.sync.drain()
        wait_clock.add_sem_waits(
            drain_inst.ins, ScopedClock({None: tick_clock.global_clock})
        )
        nc.all_engine_barrier()
        assert self.sems is not None
        popped = nc._tile_sem_poison_stack.pop()
        assert popped is self._sem_poison
        nc.clear_and_free_semaphores(list(self.sems.allocated().values()))
        nc.all_engine_barrier()
        split_multiwait(nc)

    tile.TileContext._drain_and_barrier = _patched_drain_and_barrier
    tile.TileContext._multiwait_patched = True


def _build_nc(t_steps=T, cc_mode="ar4"):
    _apply_tile_patch()
    from contextlib import ExitStack
    import concourse.bass as bass
    import concourse.tile as tile
    from concourse import mybir

    f32 = mybir.dt.float32
    bf16 = mybir.dt.bfloat16
    AF = mybir.ActivationFunctionType

    nc = bass.Bass("TRN2", target_bir_lowering=False, debug=False,
                   num_devices=N_CORES)

    G4 = 4 * H  # 2048

    xt_d = nc.dram_tensor("xt", [t_steps, 128, KX * 128], bf16, kind="ExternalInput").ap()
    wl_d = nc.dram_tensor("wl", [128, KX + KH, G4], bf16, kind="ExternalInput").ap()
    w1_d = nc.dram_tensor("w1", [128, 4, F], bf16, kind="ExternalInput").ap()
    w2_d = nc.dram_tensor("w2", [128, 8, F], bf16, kind="ExternalInput").ap()
    w3_d = nc.dram_tensor("w3", [128, 8, F], bf16, kind="ExternalInput").ap()
    w4_d = nc.dram_tensor("w4", [128, 8, 3], bf16, kind="ExternalInput").ap()
    b1_d = nc.dram_tensor("b1", [1, F], bf16, kind="ExternalInput").ap()
    b2_d = nc.dram_tensor("b2", [1, F], bf16, kind="ExternalInput").ap()
    b3_d = nc.dram_tensor("b3", [1, F], bf16, kind="ExternalInput").ap()
    b4_d = nc.dram_tensor("b4", [1, 3], bf16, kind="ExternalInput").ap()
    ones_d = nc.dram_tensor("ones", [1, 128], bf16, kind="ExternalInput").ap()
    idr_d = nc.dram_tensor("identr", [128, 128], bf16, kind="ExternalInput").ap()
    out_d = nc.dram_tensor("logitsT", [3, 128], f32, kind="ExternalOutput").ap()

    crin = nc.dram_tensor("crin", [128, F], bf16)
    crout = nc.dram_tensor("crout", [128, F], bf16)

    with tile.TileContext(nc) as tc, ExitStack() as ctx:
        wp = ctx.enter_context(tc.tile_pool(name="weights", bufs=1))
        wl_sb = wp.tile([128, KX + KH, G4], bf16, tag="wl")
        w1_sb = wp.tile([128, 4, F], bf16, tag="w1")
        w2_sb = wp.tile([128, 8, F], bf16, tag="w2")
        w3_sb = wp.tile([128, 8, F], bf16, tag="w3")
        w4_sb = wp.tile([128, 8, 3], bf16, tag="w4")
        b1_sb = wp.tile([1, F], bf16, tag="b1")
        b2_sb = wp.tile([1, F], bf16, tag="b2")
        b3_sb = wp.tile([1, F], bf16, tag="b3")
        b4_sb = wp.tile([1, 3], bf16, tag="b4")
        ones_sb = wp.tile([1, 128], bf16, tag="ones")
        idr_sb = wp.tile([128, 128], bf16, tag="idr")
        # LSTM-critical loads first, split per (chunk, bank) in the order the
        # x-part consumes them so the first matmul starts after ~128KB of
        # weight DMA instead of the full 3.5MB
        xp = ctx.enter_context(tc.tile_pool(name="xsteps", bufs=4))
        for k in range(KX):
            nc.sync.dma_start(wl_sb[:, k, :], wl_d[:, k])
        nc.sync.dma_start(idr_sb[:], idr_d[:])
        for k in range(KX, KX + KH):
            nc.sync.dma_start(wl_sb[:, k, :], wl_d[:, k])
        # MLP weights aren't needed until after the T loop — emitted late
        # (below) so their DMA doesn't delay the LSTM start.
        _mlp_loads = [(w1_sb, w1_d), (w2_sb, w2_d), (w3_sb, w3_d),
                      (w4_sb, w4_d), (b1_sb, b1_d), (b2_sb, b2_d),
                      (b3_sb, b3_d), (b4_sb, b4_d), (ones_sb, ones_d)]

        sp = ctx.enter_context(tc.tile_pool(name="state", bufs=2))

        c_prev = None
        h_pend = None     # h of step t-1, awaiting transpose (2 half tiles)
        hT_chunks = None  # hT chunk tiles for this step's h-part
        cT_bf = None

        # gate layout [i | f | o | j]; bank order = ACT completion order
        BANK_ORDER = (0, 3, 1, 2)
        GATE_FUNC = {0: AF.Sigmoid, 1: AF.Sigmoid, 2: AF.Sigmoid, 3: AF.Tanh}

        with tc.tile_pool(name="zpsum", bufs=1, space="PSUM") as zpool, \
             tc.tile_pool(name="trpsum", bufs=4, space="PSUM") as trpool:
            for t in range(t_steps):
                if t == 2:
                    for sb_t, d in _mlp_loads:
                        nc.sync.dma_start(sb_t[:], d[:])
                xt_sb = xp.tile([128, KX * 128], bf16, tag="xt")
                nc.sync.dma_start(xt_sb[:], xt_d[t])
                zb = [zpool.tile([128, 512], f32, tag=f"z{n}", name=f"zb{n}")
                      for n in range(4)]

                # x-part: bank-major in ACT completion order so the WAR wait
                # on last step's gate read is already satisfied
                for n in BANK_ORDER:
                    ns = slice(n * 512, (n + 1) * 512)
                    for k in range(KX):
                        nc.tensor.matmul(
                            zb[n][:],
                            xt_sb[:, k * 128:(k + 1) * 128],
                            wl_sb[:, k, ns],
                            start=(k == 0),
                            stop=(t == 0 and k == KX - 1),
                        )

                if h_pend is not None:
                    # transpose h_{t-1} chunk k, copy to SBUF, then step t's
                    # chunk-k h-matmuls — PE alternates transpose/matmul with
                    # no idle, consuming h halves as the tail produces them
                    hT_chunks = [
                        sp.tile([128, 128], bf16, tag=f"hT{k}", name=f"hT{k}")
                        for k in range(KH)
                    ]
                    for k in range(KH):
                        trp = trpool.tile([128, 128], bf16, tag="tr")
                        nc.tensor.transpose(
                            trp[:], h_pend[k // 2][:, (k % 2) * 128:(k % 2) * 128 + 128],
                            idr_sb[:],
                        )
                        if k % 2 == 0:
                            nc.scalar.copy(hT_chunks[k][:], trp[:])
                        else:
                            nc.vector.tensor_copy(hT_chunks[k][:], trp[:])
                        if k < 2:
                            for n in BANK_ORDER:
                                ns = slice(n * 512, (n + 1) * 512)
                                nc.tensor.matmul(
                                    zb[n][:], hT_chunks[k][:], wl_sb[:, KX + k, ns],
                                    start=False, stop=False,
                                )
                    # finish banks in ACT order for chunks 2,3
                    for n in BANK_ORDER:
                        ns = slice(n * 512, (n + 1) * 512)
                        for k in (2, 3):
                            nc.tensor.matmul(
                                zb[n][:], hT_chunks[k][:], wl_sb[:, KX + k, ns],
                                start=False, stop=(k == KH - 1),
                            )

                gates = sp.tile([128, G4], f32, tag="gates")
                for n in BANK_ORDER:
                    nc.scalar.activation(
                        gates[:, n * 512:(n + 1) * 512], zb[n][:], GATE_FUNC[n]
                    )

                # elementwise tail, two H-halves; c*f on gpsimd to keep the
                # Vector queue short (i*j, c add, h mul)
                t2 = sp.tile([128, H], f32, tag="t2")
                if t == 0:
                    c_new = t2
                else:
                    t1 = sp.tile([128, H], f32, tag="t1")
                    c_new = sp.tile([128, H], f32, tag="c")
                last = t == t_steps - 1
                if not last:
                    tanc = sp.tile([128, H], f32, tag="tanc")
                    h_new = [sp.tile([128, 256], bf16, tag=f"h{hf}",
                                     name=f"h{hf}")
                             for hf in (0, 1)]
                for hf in (0, 1):
                    sl = slice(hf * 256, (hf + 1) * 256)
                    gi = gates[:, hf * 256:hf * 256 + 256]
                    gf = gates[:, 512 + hf * 256:512 + hf * 256 + 256]
                    go = gates[:, 1024 + hf * 256:1024 + hf * 256 + 256]
                    gj = gates[:, 1536 + hf * 256:1536 + hf * 256 + 256]
                    nc.vector.tensor_mul(t2[:, sl], gi, gj)
                    if t > 0:
                        nc.gpsimd.tensor_mul(t1[:, sl], c_prev[:, sl], gf)
                        nc.vector.tensor_add(c_new[:, sl], t1[:, sl], t2[:, sl])
                    if not last:
                        nc.scalar.activation(tanc[:, sl], c_new[:, sl], AF.Tanh)
                        nc.vector.tensor_mul(h_new[hf][:], tanc[:, sl], go)
                c_prev = c_new
                if not last:
                    h_pend = h_new

            # final cell state -> bf16, transposed into cT [p, k*128+b]
            cb = sp.tile([128, H], bf16, tag="cb")
            nc.vector.tensor_copy(cb[:], c_prev[:])
            cT_bf = sp.tile([128, H], bf16, tag="cT")
            for k in range(4):
                ks = slice(k * 128, (k + 1) * 128)
                trp = trpool.tile([128, 128], bf16, tag="tr")
                nc.tensor.transpose(trp[:], cb[:, ks], idr_sb[:])
                nc.vector.tensor_copy(cT_bf[:, ks], trp[:])

        # layer-1 partial for this core's direction slice of W1, then
        # AllReduce the partial pre-activations within each batch-half group
        with tc.tile_pool(name="mlppsum", bufs=1, space="PSUM") as mp, \
             tc.tile_pool(name="mtrpsum", bufs=3, space="PSUM") as mtr:
            aps = mp.tile([128, F], f32, tag="aps1")
            for half in (0, 1):
                ms = slice(half * 512, (half + 1) * 512)
                for kc in range(4):
                    nc.tensor.matmul(
                        aps[:, ms], cT_bf[:, kc * 128:(kc + 1) * 128],
                        w1_sb[:, kc, ms], start=(kc == 0), stop=False,
                    )
                nc.tensor.matmul(aps[:, ms], ones_sb[0:1, :], b1_sb[0:1, ms],
                                 start=False, stop=True)
            part1 = sp.tile([128, F], bf16, tag="part1")
            nc.vector.tensor_copy(part1[:], aps[:])
            nc.sync.dma_start(crin.ap()[:], part1[:])
            nc.gpsimd.collective_compute(
                "AllReduce",
                mybir.AluOpType.add,
                replica_groups=[[0, 1, 2, 3], [4, 5, 6, 7]],
                ins=[crin.ap()[:]],
                outs=[crout.ap()[:]],
            )
            s1 = sp.tile([128, F], bf16, tag="s1")
            nc.sync.dma_start(s1[:], crout.ap()[:])
            act_in = sp.tile([128, F], bf16, tag="a1")
            nc.scalar.activation(act_in[:], s1[:], AF.Tanh)

            # layers 2..3 wide-N: transpose a, then psum = aT @ W + b, tanh
            for li, (w_sb, b_sb) in enumerate([(w2_sb, b2_sb), (w3_sb, b3_sb)]):
                aT = sp.tile([128, F], bf16, tag=f"aT{li}")
                for m in range(8):
                    ms = slice(m * 128, (m + 1) * 128)
                    trp = mtr.tile([128, 128], bf16, tag="mtr")
                    nc.tensor.transpose(trp[:], act_in[:, ms], idr_sb[:])
                    if m % 2 == 0:
                        nc.scalar.copy(aT[:, ms], trp[:])
                    else:
                        nc.vector.tensor_copy(aT[:, ms], trp[:])
                aps2 = mp.tile([128, F], f32, tag="apsL", name="aps2")
                for half in (0, 1):
                    ms = slice(half * 512, (half + 1) * 512)
                    for kc in range(8):
                        nc.tensor.matmul(
                            aps2[:, ms], aT[:, kc * 128:(kc + 1) * 128],
                            w_sb[:, kc, ms], start=(kc == 0), stop=False,
                        )
                    nc.tensor.matmul(aps2[:, ms], ones_sb[0:1, :], b_sb[0:1, ms],
                                     start=False, stop=True)
                nxt = sp.tile([128, F], bf16, tag=f"a{li + 2}")
                nc.scalar.activation(nxt[:], aps2[:], AF.Tanh)
                act_in = nxt

            # layer 4 in [3, batch] layout: stationary = W4 chunks
            a3T = sp.tile([128, F], bf16, tag="a3T")
            for m in range(8):
                ms = slice(m * 128, (m + 1) * 128)
                trp = mtr.tile([128, 128], bf16, tag="mtr")
                nc.tensor.transpose(trp[:], act_in[:, ms], idr_sb[:])
                if m % 2 == 0:
                    nc.scalar.copy(a3T[:, ms], trp[:])
                else:
                    nc.vector.tensor_copy(a3T[:, ms], trp[:])
            l4 = mp.tile([3, 128], f32, tag="l4")
            for kc in range(8):
                nc.tensor.matmul(
                    l4[:], w4_sb[:, kc, :], a3T[:, kc * 128:(kc + 1) * 128],
                    start=(kc == 0), stop=False,
                )
            nc.tensor.matmul(l4[:], b4_sb[0:1, :], ones_sb[0:1, :],
                             start=False, stop=True)
            lg = sp.tile([3, 128], f32, tag="lg")
            nc.scalar.copy(lg[:], l4[:])
            nc.sync.dma_start(out_d[:], lg[:])

    return nc


def _pack_core_inputs(core, inputs, t_steps=T):
    """Build the per-core in_map (numpy only)."""
    bf16 = ml_dtypes.bfloat16
    lstm = core % 4
    half = core // 4
    rows = slice(half * BC, (half + 1) * BC)

    if lstm < 2:
        x = np.asarray(inputs["premises"])[rows]
        W = np.asarray(inputs["W_fw_p"] if lstm == 0 else inputs["W_bw_p"])
        b = np.asarray(inputs["b_fw_p"] if lstm == 0 else inputs["b_bw_p"])
    else:
        x = np.asarray(inputs["hypotheses"])[rows]
        W = np.asarray(inputs["W_fw_h"] if lstm == 2 else inputs["W_bw_h"])
        b = np.asarray(inputs["b_fw_h"] if lstm == 2 else inputs["b_bw_h"])
    x = x[:, :t_steps]
    if lstm % 2 == 1:
        x = x[:, ::-1, :]

    # gate reorder [i, f, o, j]; fold forget_bias=1.0 into b
    perm = np.concatenate([
        np.arange(0, H), np.arange(2 * H, 3 * H),
        np.arange(3 * H, 4 * H), np.arange(H, 2 * H),
    ])
    Wp = W[:, perm].astype(np.float32)
    bp = b[perm].astype(np.float32).copy()
    bp[H:2 * H] += 1.0  # forget gate slice in new layout

    xa = np.zeros((BC, t_steps, EP), np.float32)
    xa[:, :, :E] = x
    xa[:, :, E] = 1.0
    xt = np.ascontiguousarray(
        xa.reshape(BC, t_steps, KX, 128).transpose(1, 3, 2, 0)
    ).reshape(t_steps, 128, KX * 128)

    wl = np.zeros((128, KX + KH, 4 * H), np.float32)
    W_aug_x = np.zeros((EP, 4 * H), np.float32)
    W_aug_x[:E] = Wp[:E]
    W_aug_x[E] = bp
    for k in range(KX):
        wl[:, k, :] = W_aug_x[k * 128:(k + 1) * 128]
    for k in range(KH):
        wl[:, KX + k, :] = Wp[E + k * 128: E + (k + 1) * 128]

    W1 = np.asarray(inputs["W1"]).astype(np.float32)
    W2 = np.asarray(inputs["W2"]).astype(np.float32)
    W3 = np.asarray(inputs["W3"]).astype(np.float32)
    W4 = np.asarray(inputs["W4"]).astype(np.float32)
    # per-core W1 slice: rows for this core's direction in the rnn concat
    w1s = W1[512 * lstm:512 * (lstm + 1)].reshape(4, 128, F).transpose(1, 0, 2)
    w2 = W2.reshape(8, 128, F).transpose(1, 0, 2).astype(bf16)
    w3 = W3.reshape(8, 128, F).transpose(1, 0, 2).astype(bf16)
    w4 = W4.reshape(8, 128, 3).transpose(1, 0, 2).astype(bf16)

    return {
        "xt": xt.astype(bf16),
        "wl": wl.astype(bf16),
        "w1": np.ascontiguousarray(w1s).astype(bf16),
        "w2": np.ascontiguousarray(w2),
        "w3": np.ascontiguousarray(w3),
        "w4": np.ascontiguousarray(w4),
        # b1 scaled by 1/4: each of the 4 group cores adds it once into the
        # AllReduce sum
        "b1": (np.asarray(inputs["b1"]).reshape(1, F) * 0.25).astype(bf16),
        "b2": np.asarray(inputs["b2"]).reshape(1, F).astype(bf16),
        "b3": np.asarray(inputs["b3"]).reshape(1, F).astype(bf16),
        "b4": np.asarray(inputs["b4"]).reshape(1, 3).astype(bf16),
        "ones": np.ones((1, 128), bf16),
        "identr": np.eye(128, dtype=bf16),
    }


def _install_ntff_shim():
    """This image's `antenv` lacks `axon_hooks`; provide it so
    run_bass_kernel_spmd(trace=True) can capture NTFF profiles."""
    import sys
    import types

    if "antenv.axon_hooks" in sys.modules:
        return
    mod = types.ModuleType("antenv.axon_hooks")
    state = {"hook": None}
    mod.set_axon_ntff_profile_hook = lambda h: state.__setitem__("hook", h)
    mod.get_axon_ntff_profile_hook = lambda: state["hook"]
    sys.modules["antenv.axon_hooks"] = mod
    try:
        from trn_agent_boot.trn_boot import _ntff_profile_via_ctypes

        mod.set_axon_ntff_profile_hook(
            _ntff_profile_via_ctypes("/opt/axon/libaxon_pjrt.so")
        )
    except Exception:
        pass


def _run(inputs, trace=False, t_steps=T, ldt="bf16"):
    if trace:
        _install_ntff_shim()
    from concourse.bass_utils import run_bass_kernel_spmd

    key = (t_steps,)
    if key not in _cache:
        _cache[key] = _build_nc(t_steps)
    nc = _cache[key]
    in_maps = [_pack_core_inputs(c, inputs, t_steps) for c in range(N_CORES)]
    res = run_bass_kernel_spmd(
        nc, in_maps, list(range(N_CORES)), trace=trace
    )
    out = np.zeros((B, 3), np.float32)
    out[0:BC] = res.results[0]["logitsT"].T
    out[BC:2 * BC] = res.results[4]["logitsT"].T
    return out, res


def kernel(**inputs) -> np.ndarray:
    out, _ = _run(inputs, trace=False)
    return out


# revision 18
# speedup vs baseline: 1.0304x; 1.0124x over previous
"""BiRNN (Bowman SNLI) Trainium2 kernel.

Sharding: 8 cores = 4 LSTM directions x 2 batch halves (SPMD — same program,
per-core weights/inputs differ). Each core runs one LSTM (batch 128, T=128):
per step z = [x_t, h, 1] @ W_aug accumulated in PSUM with stationary =
x_t^T / h^T chunks and moving = weight columns (N=512, bf16). Gates are
column-reordered [i, f, o, j] host-side and the forget bias folded into b.

Per-step emission order keeps the PE busy: x-part of step t, then the four
h^T transposes of step t-1 interleaved with step t's h-part matmuls so the
recurrence never stalls the tensor engine. The elementwise tail is spread
over Vector (i*j, c, h), GpSimd (c*f), and Scalar (gates/tanh + hT copies).

The MLP is K-sharded: each core computes its direction's slice of layer-1
pre-activations, a 4-core AllReduce sums them, and every core then runs
layers 2-4 redundantly in wide-N layout (PE transposes between layers).
Cores 0 and 4 write logits^T [3, 128] which the host reads.
"""
import numpy as np
import ml_dtypes

# Harness-visible constants
B, T, E, H, F = 256, 128, 300, 512, 1024
BC = 128          # batch per core
N_CORES = 8
EP = 384          # padded x feature dim (300 x + 1 bias + pad)
KX = 3            # x stationary chunks (last has 45 valid rows)
KH = 4            # h stationary chunks

_cache = {}


def _apply_tile_patch():
    """walrus here allows ONE semaphore wait per instruction; Tile's tail
    drain (and occasionally other instructions) get more. Split extra waits
    onto same-engine NoOp carriers inserted immediately before."""
    import concourse.tile as tile
    import concourse.mybir as mybir
    from concourse.tile import ScopedClock

    if getattr(tile.TileContext, "_multiwait_patched", False):
        return

    def split_multiwait(nc):
        for f in nc.m.functions:
            for bb in f.blocks:
                insts = bb.instructions
                if not any(
                    i.sync_info is not None and len(i.sync_info.on_wait) > 1
                    for i in insts
                ):
                    continue
                new = []
                for inst in insts:
                    si = inst.sync_info
                    if si is not None and len(si.on_wait) > 1:
                        waits = list(si.on_wait)
                        for w in waits[:-1]:
                            carrier = mybir.InstNoOp(
                                name=nc.get_next_instruction_name(), ins=[], outs=[]
                            )
                            carrier.engine = inst.engine
                            carrier.sync_info = mybir.SyncInfo(
                                on_wait=[w], on_update=[]
                            )
                            nc.register_instruction(carrier, overwrite=True)
                            new.append(carrier)
                        si.on_wait = [waits[-1]]
                    new.append(inst)
                bb.instructions = new

    def _patched_drain_and_barrier(self, tick_clock, wait_clock):
        nc = self.nc
        drain_inst = nc.sync.drain()
        wait_clock.add_sem_waits(
            drain_inst.ins, ScopedClock({None: tick_clock.global_clock})
        )
        nc.all_engine_barrier()
        assert self.sems is not None
        popped = nc._tile_sem_poison_stack.pop()
        assert popped is self._sem_poison
        nc.clear_and_free_semaphores(list(self.sems.allocated().values()))
        nc.all_engine_barrier()
        split_multiwait(nc)

    tile.TileContext._drain_and_barrier = _patched_drain_and_barrier
    tile.TileContext._multiwait_patched = True


def _build_nc(t_steps=T, cc_mode="ar4"):
    _apply_tile_patch()
    from contextlib import ExitStack
    import concourse.bass as bass
    import concourse.tile as tile
    from concourse import mybir

    f32 = mybir.dt.float32
    bf16 = mybir.dt.bfloat16
    AF = mybir.ActivationFunctionType

    nc = bass.Bass("TRN2", target_bir_lowering=False, debug=False,
                   num_devices=N_CORES)

    G4 = 4 * H  # 2048

    xt_d = nc.dram_tensor("xt", [t_steps, 128, KX * 128], bf16, kind="ExternalInput").ap()
    wl_d = nc.dram_tensor("wl", [128, KX + KH, G4], bf16, kind="ExternalInput").ap()
    w1_d = nc.dram_tensor("w1", [128, 4, F], bf16, kind="ExternalInput").ap()
    w2_d = nc.dram_tensor("w2", [128, 8, F], bf16, kind="ExternalInput").ap()
    w3_d = nc.dram_tensor("w3", [128, 8, F], bf16, kind="ExternalInput").ap()
    w4_d = nc.dram_tensor("w4", [128, 8, 3], bf16, kind="ExternalInput").ap()
    b1_d = nc.dram_tensor("b1", [1, F], bf16, kind="ExternalInput").ap()
    b2_d = nc.dram_tensor("b2", [1, F], bf16, kind="ExternalInput").ap()
    b3_d = nc.dram_tensor("b3", [1, F], bf16, kind="ExternalInput").ap()
    b4_d = nc.dram_tensor("b4", [1, 3], bf16, kind="ExternalInput").ap()
    ones_d = nc.dram_tensor("ones", [1, 128], bf16, kind="ExternalInput").ap()
    idr_d = nc.dram_tensor("identr", [128, 128], bf16, kind="ExternalInput").ap()
    out_d = nc.dram_tensor("logitsT", [3, 128], f32, kind="ExternalOutput").ap()

    crin = nc.dram_tensor("crin", [128, F], bf16)
    crout = nc.dram_tensor("crout", [128, F], bf16)

    with tile.TileContext(nc) as tc, ExitStack() as ctx:
        wp = ctx.enter_context(tc.tile_pool(name="weights", bufs=1))
        wl_sb = wp.tile([128, KX + KH, G4], bf16, tag="wl")
        w1_sb = wp.tile([128, 4, F], bf16, tag="w1")
        w2_sb = wp.tile([128, 8, F], bf16, tag="w2")
        w3_sb = wp.tile([128, 8, F], bf16, tag="w3")
        w4_sb = wp.tile([128, 8, 3], bf16, tag="w4")
        b1_sb = wp.tile([1, F], bf16, tag="b1")
        b2_sb = wp.tile([1, F], bf16, tag="b2")
        b3_sb = wp.tile([1, F], bf16, tag="b3")
        b4_sb = wp.tile([1, 3], bf16, tag="b4")
        ones_sb = wp.tile([1, 128], bf16, tag="ones")
        idr_sb = wp.tile([128, 128], bf16, tag="idr")
        # LSTM-critical loads first, split per (chunk, bank) in the order the
        # x-part consumes them so the first matmul starts after ~128KB of
        # weight DMA instead of the full 3.5MB
        xp = ctx.enter_context(tc.tile_pool(name="xsteps", bufs=4))
        for k in range(KX):
            nc.sync.dma_start(wl_sb[:, k, :], wl_d[:, k])
        nc.sync.dma_start(idr_sb[:], idr_d[:])
        for k in range(KX, KX + KH):
            nc.sync.dma_start(wl_sb[:, k, :], wl_d[:, k])
        # MLP weights aren't needed until after the T loop — emitted late
        # (below) so their DMA doesn't delay the LSTM start.
        _mlp_loads = [(w1_sb, w1_d), (w2_sb, w2_d), (w3_sb, w3_d),
                      (w4_sb, w4_d), (b1_sb, b1_d), (b2_sb, b2_d),
                      (b3_sb, b3_d), (b4_sb, b4_d), (ones_sb, ones_d)]

        sp = ctx.enter_context(tc.tile_pool(name="state", bufs=2))

        c_prev = None
        h_pend = None     # h of step t-1, awaiting transpose (2 half tiles)
        hT_chunks = None  # hT chunk tiles for this step's h-part
        cT_bf = None

        # gate layout [i | f | o | j]; bank order = ACT completion order
        BANK_ORDER = (0, 3, 1, 2)
        GATE_FUNC = {0: AF.Sigmoid, 1: AF.Sigmoid, 2: AF.Sigmoid, 3: AF.Tanh}

        with tc.tile_pool(name="zpsum", bufs=1, space="PSUM") as zpool, \
             tc.tile_pool(name="trpsum", bufs=4, space="PSUM") as trpool:
            for t in range(t_steps):
                if t == 2:
                    for sb_t, d in _mlp_loads:
                        nc.sync.dma_start(sb_t[:], d[:])
                xt_sb = xp.tile([128, KX * 128], bf16, tag="xt")
                nc.sync.dma_start(xt_sb[:], xt_d[t])
                zb = [zpool.tile([128, 512], f32, tag=f"z{n}", name=f"zb{n}")
                      for n in range(4)]

                # x-part: bank-major in ACT completion order so the WAR wait
                # on last step's gate read is already satisfied
                for n in BANK_ORDER:
                    ns = slice(n * 512, (n + 1) * 512)
                    for k in range(KX):
                        nc.tensor.matmul(
                            zb[n][:],
                            xt_sb[:, k * 128:(k + 1) * 128],
                            wl_sb[:, k, ns],
                            start=(k == 0),
                            stop=(t == 0 and k == KX - 1),
                        )

                if h_pend is not None:
                    # transpose h_{t-1} chunk k, copy to SBUF, then step t's
                    # chunk-k h-matmuls — PE alternates transpose/matmul with
                    # no idle, consuming h halves as the tail produces them
                    hT_chunks = [
                        sp.tile([128, 128], bf16, tag=f"hT{k}", name=f"hT{k}")
                        for k in range(KH)
                    ]
                    for k in range(KH):
                        trp = trpool.tile([128, 128], bf16, tag="tr")
                        nc.tensor.transpose(
                            trp[:], h_pend[k // 2][:, (k % 2) * 128:(k % 2) * 128 + 128],
                            idr_sb[:],
                        )
                        if k % 2 == 0:
                            nc.scalar.copy(hT_chunks[k][:], trp[:])
                        else:
                            nc.vector.tensor_copy(hT_chunks[k][:], trp[:])
                        if k < 2:
                            for n in BANK_ORDER:
                                ns = slice(n * 512, (n + 1) * 512)
                                nc.tensor.matmul(
                                    zb[n][:], hT_chunks[k][:], wl_sb[:, KX + k, ns],
                                    start=False, stop=False,
                                )
                    # finish banks in ACT order for chunks 2,3
                    for n in BANK_ORDER:
                        ns = slice(n * 512, (n + 1) * 512)
                        for k in (2, 3):
                            nc.tensor.matmul(
                                zb[n][:], hT_chunks[k][:], wl_sb[:, KX + k, ns],
                                start=False, stop=(k == KH - 1),
                            )

                gates = sp.tile([128, G4], bf16, tag="gates")
                for n in BANK_ORDER:
                    nc.scalar.activation(
                        gates[:, n * 512:(n + 1) * 512], zb[n][:], GATE_FUNC[n]
                    )

                # elementwise tail, two H-halves; c*f on gpsimd to keep the
                # Vector queue short (i*j, c add, h mul)
                t2 = sp.tile([128, H], f32, tag="t2")
                if t == 0:
                    c_new = t2
                else:
                    t1 = sp.tile([128, H], f32, tag="t1")
                    c_new = sp.tile([128, H], f32, tag="c")
                last = t == t_steps - 1
                if not last:
                    tanc = sp.tile([128, H], bf16, tag="tanc")
                    h_new = [sp.tile([128, 256], bf16, tag=f"h{hf}",
                                     name=f"h{hf}")
                             for hf in (0, 1)]
                for hf in (0, 1):
                    sl = slice(hf * 256, (hf + 1) * 256)
                    gi = gates[:, hf * 256:hf * 256 + 256]
                    gf = gates[:, 512 + hf * 256:512 + hf * 256 + 256]
                    go = gates[:, 1024 + hf * 256:1024 + hf * 256 + 256]
                    gj = gates[:, 1536 + hf * 256:1536 + hf * 256 + 256]
                    nc.vector.tensor_mul(t2[:, sl], gi, gj)
                    if t > 0:
                        nc.gpsimd.tensor_mul(t1[:, sl], c_prev[:, sl], gf)
                        nc.vector.tensor_add(c_new[:, sl], t1[:, sl], t2[:, sl])
                    if not last:
                        nc.scalar.activation(tanc[:, sl], c_new[:, sl], AF.Tanh)
                        nc.vector.tensor_mul(h_new[hf][:], tanc[:, sl], go)
                c_prev = c_new
                if not last:
                    h_pend = h_new

            # final cell state -> bf16, transposed into cT [p, k*128+b]
            cb = sp.tile([128, H], bf16, tag="cb")
            nc.vector.tensor_copy(cb[:], c_prev[:])
            cT_bf = sp.tile([128, H], bf16, tag="cT")
            for k in range(4):
                ks = slice(k * 128, (k + 1) * 128)
                trp = trpool.tile([128, 128], bf16, tag="tr")
                nc.tensor.transpose(trp[:], cb[:, ks], idr_sb[:])
                nc.vector.tensor_copy(cT_bf[:, ks], trp[:])

        # layer-1 partial for this core's direction slice of W1, then
        # AllReduce the partial pre-activations within each batch-half group
        with tc.tile_pool(name="mlppsum", bufs=1, space="PSUM") as mp, \
             tc.tile_pool(name="mtrpsum", bufs=3, space="PSUM") as mtr:
            aps = mp.tile([128, F], f32, tag="aps1")
            for half in (0, 1):
                ms = slice(half * 512, (half + 1) * 512)
                for kc in range(4):
                    nc.tensor.matmul(
                        aps[:, ms], cT_bf[:, kc * 128:(kc + 1) * 128],
                        w1_sb[:, kc, ms], start=(kc == 0), stop=False,
                    )
                nc.tensor.matmul(aps[:, ms], ones_sb[0:1, :], b1_sb[0:1, ms],
                                 start=False, stop=True)
            part1 = sp.tile([128, F], bf16, tag="part1")
            nc.vector.tensor_copy(part1[:], aps[:])
            nc.sync.dma_start(crin.ap()[:], part1[:])
            nc.gpsimd.collective_compute(
                "AllReduce",
                mybir.AluOpType.add,
                replica_groups=[[0, 1, 2, 3], [4, 5, 6, 7]],
                ins=[crin.ap()[:]],
                outs=[crout.ap()[:]],
            )
            # keep the PE p-state hot through the ~24us collective so the
            # MLP matmuls run at full clock (idle PE drops to ~1.6GHz)
            warm = mp.tile([128, 512], f32, tag="apsL", name="warm")
            for _ in range(64):
                nc.tensor.matmul(warm[:], idr_sb[:], wl_sb[:, 0, 0:512],
                                 start=True, stop=True)
            s1 = sp.tile([128, F], bf16, tag="s1")
            nc.sync.dma_start(s1[:], crout.ap()[:])
            act_in = sp.tile([128, F], bf16, tag="a1")
            nc.scalar.activation(act_in[:], s1[:], AF.Tanh)

            # layers 2..3 wide-N: transpose a, then psum = aT @ W + b, tanh
            for li, (w_sb, b_sb) in enumerate([(w2_sb, b2_sb), (w3_sb, b3_sb)]):
                aT = sp.tile([128, F], bf16, tag=f"aT{li}")
                for m in range(8):
                    ms = slice(m * 128, (m + 1) * 128)
                    trp = mtr.tile([128, 128], bf16, tag="mtr")
                    nc.tensor.transpose(trp[:], act_in[:, ms], idr_sb[:])
                    if m % 2 == 0:
                        nc.scalar.copy(aT[:, ms], trp[:])
                    else:
                        nc.vector.tensor_copy(aT[:, ms], trp[:])
                aps2 = mp.tile([128, F], f32, tag="apsL", name="aps2")
                for half in (0, 1):
                    ms = slice(half * 512, (half + 1) * 512)
                    for kc in range(8):
                        nc.tensor.matmul(
                            aps2[:, ms], aT[:, kc * 128:(kc + 1) * 128],
                            w_sb[:, kc, ms], start=(kc == 0), stop=False,
                        )
                    nc.tensor.matmul(aps2[:, ms], ones_sb[0:1, :], b_sb[0:1, ms],
                                     start=False, stop=True)
                nxt = sp.tile([128, F], bf16, tag=f"a{li + 2}")
                nc.scalar.activation(nxt[:], aps2[:], AF.Tanh)
                act_in = nxt

            # layer 4 in [3, batch] layout: stationary = W4 chunks
            a3T = sp.tile([128, F], bf16, tag="a3T")
            for m in range(8):
                ms = slice(m * 128, (m + 1) * 128)
                trp = mtr.tile([128, 128], bf16, tag="mtr")
                nc.tensor.transpose(trp[:], act_in[:, ms], idr_sb[:])
                if m % 2 == 0:
                    nc.scalar.copy(a3T[:, ms], trp[:])
                else:
                    nc.vector.tensor_copy(a3T[:, ms], trp[:])
            l4 = mp.tile([3, 128], f32, tag="l4")
            for kc in range(8):
                nc.tensor.matmul(
                    l4[:], w4_sb[:, kc, :], a3T[:, kc * 128:(kc + 1) * 128],
                    start=(kc == 0), stop=False,
                )
            nc.tensor.matmul(l4[:], b4_sb[0:1, :], ones_sb[0:1, :],
                             start=False, stop=True)
            lg = sp.tile([3, 128], f32, tag="lg")
            nc.scalar.copy(lg[:], l4[:])
            nc.sync.dma_start(out_d[:], lg[:])

    return nc


def _pack_core_inputs(core, inputs, t_steps=T):
    """Build the per-core in_map (numpy only)."""
    bf16 = ml_dtypes.bfloat16
    lstm = core % 4
    half = core // 4
    rows = slice(half * BC, (half + 1) * BC)

    if lstm < 2:
        x = np.asarray(inputs["premises"])[rows]
        W = np.asarray(inputs["W_fw_p"] if lstm == 0 else inputs["W_bw_p"])
        b = np.asarray(inputs["b_fw_p"] if lstm == 0 else inputs["b_bw_p"])
    else:
        x = np.asarray(inputs["hypotheses"])[rows]
        W = np.asarray(inputs["W_fw_h"] if lstm == 2 else inputs["W_bw_h"])
        b = np.asarray(inputs["b_fw_h"] if lstm == 2 else inputs["b_bw_h"])
    x = x[:, :t_steps]
    if lstm % 2 == 1:
        x = x[:, ::-1, :]

    # gate reorder [i, f, o, j]; fold forget_bias=1.0 into b
    perm = np.concatenate([
        np.arange(0, H), np.arange(2 * H, 3 * H),
        np.arange(3 * H, 4 * H), np.arange(H, 2 * H),
    ])
    Wp = W[:, perm].astype(np.float32)
    bp = b[perm].astype(np.float32).copy()
    bp[H:2 * H] += 1.0  # forget gate slice in new layout

    xa = np.zeros((BC, t_steps, EP), np.float32)
    xa[:, :, :E] = x
    xa[:, :, E] = 1.0
    xt = np.ascontiguousarray(
        xa.reshape(BC, t_steps, KX, 128).transpose(1, 3, 2, 0)
    ).reshape(t_steps, 128, KX * 128)

    wl = np.zeros((128, KX + KH, 4 * H), np.float32)
    W_aug_x = np.zeros((EP, 4 * H), np.float32)
    W_aug_x[:E] = Wp[:E]
    W_aug_x[E] = bp
    for k in range(KX):
        wl[:, k, :] = W_aug_x[k * 128:(k + 1) * 128]
    for k in range(KH):
        wl[:, KX + k, :] = Wp[E + k * 128: E + (k + 1) * 128]

    W1 = np.asarray(inputs["W1"]).astype(np.float32)
    W2 = np.asarray(inputs["W2"]).astype(np.float32)
    W3 = np.asarray(inputs["W3"]).astype(np.float32)
    W4 = np.asarray(inputs["W4"]).astype(np.float32)
    # per-core W1 slice: rows for this core's direction in the rnn concat
    w1s = W1[512 * lstm:512 * (lstm + 1)].reshape(4, 128, F).transpose(1, 0, 2)
    w2 = W2.reshape(8, 128, F).transpose(1, 0, 2).astype(bf16)
    w3 = W3.reshape(8, 128, F).transpose(1, 0, 2).astype(bf16)
    w4 = W4.reshape(8, 128, 3).transpose(1, 0, 2).astype(bf16)

    return {
        "xt": xt.astype(bf16),
        "wl": wl.astype(bf16),
        "w1": np.ascontiguousarray(w1s).astype(bf16),
        "w2": np.ascontiguousarray(w2),
        "w3": np.ascontiguousarray(w3),
        "w4": np.ascontiguousarray(w4),
        # b1 scaled by 1/4: each of the 4 group cores adds it once into the
        # AllReduce sum
        "b1": (np.asarray(inputs["b1"]).reshape(1, F) * 0.25).astype(bf16),
        "b2": np.asarray(inputs["b2"]).reshape(1, F).astype(bf16),
        "b3": np.asarray(inputs["b3"]).reshape(1, F).astype(bf16),
        "b4": np.asarray(inputs["b4"]).reshape(1, 3).astype(bf16),
        "ones": np.ones((1, 128), bf16),
        "identr": np.eye(128, dtype=bf16),
    }


def _install_ntff_shim():
    """This image's `antenv` lacks `axon_hooks`; provide it so
    run_bass_kernel_spmd(trace=True) can capture NTFF profiles."""
    import sys
    import types

    if "antenv.axon_hooks" in sys.modules:
        return
    mod = types.ModuleType("antenv.axon_hooks")
    state = {"hook": None}
    mod.set_axon_ntff_profile_hook = lambda h: state.__setitem__("hook", h)
    mod.get_axon_ntff_profile_hook = lambda: state["hook"]
    sys.modules["antenv.axon_hooks"] = mod
    try:
        from trn_agent_boot.trn_boot import _ntff_profile_via_ctypes

        mod.set_axon_ntff_profile_hook(
            _ntff_profile_via_ctypes("/opt/axon/libaxon_pjrt.so")
        )
    except Exception:
        pass


def _run(inputs, trace=False, t_steps=T, ldt="bf16"):
    if trace:
        _install_ntff_shim()
    from concourse.bass_utils import run_bass_kernel_spmd

    key = (t_steps,)
    if key not in _cache:
        _cache[key] = _build_nc(t_steps)
    nc = _cache[key]
    in_maps = [_pack_core_inputs(c, inputs, t_steps) for c in range(N_CORES)]
    res = run_bass_kernel_spmd(
        nc, in_maps, list(range(N_CORES)), trace=trace
    )
    out = np.zeros((B, 3), np.float32)
    out[0:BC] = res.results[0]["logitsT"].T
    out[BC:2 * BC] = res.results[4]["logitsT"].T
    return out, res


def kernel(**inputs) -> np.ndarray:
    out, _ = _run(inputs, trace=False)
    return out


# revision 28
# speedup vs baseline: 1.0417x; 1.0110x over previous
"""BiRNN (Bowman SNLI) Trainium2 kernel.

Sharding: 8 cores = 4 LSTM directions x 2 batch halves (SPMD — same program,
per-core weights/inputs differ). Each core runs one LSTM (batch 128, T=128):
per step z = [x_t, h, 1] @ W_aug accumulated in PSUM with stationary =
x_t^T / h^T chunks and moving = weight columns (N=512, bf16). Gates are
column-reordered [i, f, o, j] host-side and the forget bias folded into b.

Per-step emission order keeps the PE busy: x-part of step t, then the four
h^T transposes of step t-1 interleaved with step t's h-part matmuls so the
recurrence never stalls the tensor engine. The elementwise tail is spread
over Vector (i*j, c, h), GpSimd (c*f), and Scalar (gates/tanh + hT copies).

The MLP is K-sharded: each core computes its direction's slice of layer-1
pre-activations, a 4-core AllReduce sums them, and every core then runs
layers 2-4 redundantly in wide-N layout (PE transposes between layers).
Cores 0 and 4 write logits^T [3, 128] which the host reads.
"""
import numpy as np
import ml_dtypes

# Harness-visible constants
B, T, E, H, F = 256, 128, 300, 512, 1024
BC = 128          # batch per core
N_CORES = 8
EP = 384          # padded x feature dim (300 x + 1 bias + pad)
KX = 3            # x stationary chunks (last has 45 valid rows)
KH = 4            # h stationary chunks

_cache = {}


def _apply_tile_patch():
    """walrus here allows ONE semaphore wait per instruction; Tile's tail
    drain (and occasionally other instructions) get more. Split extra waits
    onto same-engine NoOp carriers inserted immediately before."""
    import concourse.tile as tile
    import concourse.mybir as mybir
    from concourse.tile import ScopedClock

    if getattr(tile.TileContext, "_multiwait_patched", False):
        return

    def split_multiwait(nc):
        for f in nc.m.functions:
            for bb in f.blocks:
                insts = bb.instructions
                if not any(
                    i.sync_info is not None and len(i.sync_info.on_wait) > 1
                    for i in insts
                ):
                    continue
                new = []
                for inst in insts:
                    si = inst.sync_info
                    if si is not None and len(si.on_wait) > 1:
                        waits = list(si.on_wait)
                        for w in waits[:-1]:
                            carrier = mybir.InstNoOp(
                                name=nc.get_next_instruction_name(), ins=[], outs=[]
                            )
                            carrier.engine = inst.engine
                            carrier.sync_info = mybir.SyncInfo(
                                on_wait=[w], on_update=[]
                            )
                            nc.register_instruction(carrier, overwrite=True)
                            new.append(carrier)
                        si.on_wait = [waits[-1]]
                    new.append(inst)
                bb.instructions = new

    def _patched_drain_and_barrier(self, tick_clock, wait_clock):
        nc = self.nc
        drain_inst = nc.sync.drain()
        wait_clock.add_sem_waits(
            drain_inst.ins, ScopedClock({None: tick_clock.global_clock})
        )
        nc.all_engine_barrier()
        assert self.sems is not None
        popped = nc._tile_sem_poison_stack.pop()
        assert popped is self._sem_poison
        nc.clear_and_free_semaphores(list(self.sems.allocated().values()))
        nc.all_engine_barrier()
        split_multiwait(nc)

    tile.TileContext._drain_and_barrier = _patched_drain_and_barrier
    tile.TileContext._multiwait_patched = True


def _build_nc(t_steps=T, cc_mode="ar4"):
    _apply_tile_patch()
    from contextlib import ExitStack
    import concourse.bass as bass
    import concourse.tile as tile
    from concourse import mybir

    f32 = mybir.dt.float32
    bf16 = mybir.dt.bfloat16
    AF = mybir.ActivationFunctionType

    nc = bass.Bass("TRN2", target_bir_lowering=False, debug=False,
                   num_devices=N_CORES)

    G4 = 4 * H  # 2048

    xt_d = nc.dram_tensor("xt", [t_steps, 128, KX * 128], bf16, kind="ExternalInput").ap()
    wl_d = nc.dram_tensor("wl", [128, KX + KH, G4], bf16, kind="ExternalInput").ap()
    w1_d = nc.dram_tensor("w1", [128, 4, F], bf16, kind="ExternalInput").ap()
    w2_d = nc.dram_tensor("w2", [128, 8, F], bf16, kind="ExternalInput").ap()
    w3_d = nc.dram_tensor("w3", [128, 8, F], bf16, kind="ExternalInput").ap()
    w4_d = nc.dram_tensor("w4", [128, 8, 3], bf16, kind="ExternalInput").ap()
    b1_d = nc.dram_tensor("b1", [1, F], bf16, kind="ExternalInput").ap()
    b2_d = nc.dram_tensor("b2", [1, F], bf16, kind="ExternalInput").ap()
    b3_d = nc.dram_tensor("b3", [1, F], bf16, kind="ExternalInput").ap()
    b4_d = nc.dram_tensor("b4", [1, 3], bf16, kind="ExternalInput").ap()
    ones_d = nc.dram_tensor("ones", [1, 128], bf16, kind="ExternalInput").ap()
    m01_d = nc.dram_tensor("m01", [128, 2], f32, kind="ExternalInput").ap()
    idr_d = nc.dram_tensor("identr", [128, 128], bf16, kind="ExternalInput").ap()
    out_d = nc.dram_tensor("logitsT", [3, 128], f32, kind="ExternalOutput").ap()

    crin = nc.dram_tensor("crin", [2, 128, F], bf16)
    crout = nc.dram_tensor("crout", [2, 128, F], bf16, addr_space="Shared")

    with tile.TileContext(nc) as tc, ExitStack() as ctx:
        wp = ctx.enter_context(tc.tile_pool(name="weights", bufs=1))
        wl_sb = wp.tile([128, KX + KH, G4], bf16, tag="wl")
        w1_sb = wp.tile([128, 4, F], bf16, tag="w1")
        w2_sb = wp.tile([128, 8, F], bf16, tag="w2")
        w3_sb = wp.tile([128, 8, F], bf16, tag="w3")
        w4_sb = wp.tile([128, 8, 3], bf16, tag="w4")
        b1_sb = wp.tile([1, F], bf16, tag="b1")
        b2_sb = wp.tile([1, F], bf16, tag="b2")
        b3_sb = wp.tile([1, F], bf16, tag="b3")
        b4_sb = wp.tile([1, 3], bf16, tag="b4")
        ones_sb = wp.tile([1, 128], bf16, tag="ones")
        m01_sb = wp.tile([128, 2], f32, tag="m01")
        idr_sb = wp.tile([128, 128], bf16, tag="idr")
        # LSTM-critical loads first, split per (chunk, bank) in the order the
        # x-part consumes them so the first matmul starts after ~128KB of
        # weight DMA instead of the full 3.5MB
        xp = ctx.enter_context(tc.tile_pool(name="xsteps", bufs=4))
        for k in range(KX):
            nc.sync.dma_start(wl_sb[:, k, :], wl_d[:, k])
        nc.sync.dma_start(idr_sb[:], idr_d[:])
        for k in range(KX, KX + KH):
            nc.sync.dma_start(wl_sb[:, k, :], wl_d[:, k])
        # MLP weights aren't needed until after the T loop — emitted late
        # (below) so their DMA doesn't delay the LSTM start.
        _mlp_loads = [(w1_sb, w1_d), (w2_sb, w2_d), (w3_sb, w3_d),
                      (w4_sb, w4_d), (b1_sb, b1_d), (b2_sb, b2_d),
                      (b3_sb, b3_d), (b4_sb, b4_d), (ones_sb, ones_d),
                      (m01_sb, m01_d)]

        sp = ctx.enter_context(tc.tile_pool(name="state", bufs=2))

        c_prev = None
        h_pend = None     # h of step t-1, awaiting transpose (2 half tiles)
        hT_chunks = None  # hT chunk tiles for this step's h-part
        cT_bf = None

        # gate layout [i | f | o | j]; bank order = ACT completion order
        BANK_ORDER = (0, 3, 1, 2)
        GATE_FUNC = {0: AF.Sigmoid, 1: AF.Sigmoid, 2: AF.Sigmoid, 3: AF.Tanh}

        with tc.tile_pool(name="zpsum", bufs=1, space="PSUM") as zpool, \
             tc.tile_pool(name="trpsum", bufs=4, space="PSUM") as trpool:
            for t in range(t_steps):
                if t == 2:
                    for sb_t, d in _mlp_loads:
                        nc.sync.dma_start(sb_t[:], d[:])
                xt_sb = xp.tile([128, KX * 128], bf16, tag="xt")
                nc.sync.dma_start(xt_sb[:], xt_d[t])
                zb = [zpool.tile([128, 512], f32, tag=f"z{n}", name=f"zb{n}")
                      for n in range(4)]

                # x-part: bank-major in ACT completion order so the WAR wait
                # on last step's gate read is already satisfied
                for n in BANK_ORDER:
                    ns = slice(n * 512, (n + 1) * 512)
                    for k in range(KX):
                        nc.tensor.matmul(
                            zb[n][:],
                            xt_sb[:, k * 128:(k + 1) * 128],
                            wl_sb[:, k, ns],
                            start=(k == 0),
                            stop=(t == 0 and k == KX - 1),
                        )

                if h_pend is not None:
                    # transpose h_{t-1} chunk k, copy to SBUF, then step t's
                    # chunk-k h-matmuls — PE alternates transpose/matmul with
                    # no idle, consuming h halves as the tail produces them
                    hT_chunks = [
                        sp.tile([128, 128], bf16, tag=f"hT{k}", name=f"hT{k}")
                        for k in range(KH)
                    ]
                    for k in range(KH):
                        trp = trpool.tile([128, 128], bf16, tag="tr")
                        nc.tensor.transpose(
                            trp[:], h_pend[k // 2][:, (k % 2) * 128:(k % 2) * 128 + 128],
                            idr_sb[:],
                        )
                        if k % 2 == 0:
                            nc.scalar.copy(hT_chunks[k][:], trp[:])
                        else:
                            nc.vector.tensor_copy(hT_chunks[k][:], trp[:])
                        if k < 2:
                            for n in BANK_ORDER:
                                ns = slice(n * 512, (n + 1) * 512)
                                nc.tensor.matmul(
                                    zb[n][:], hT_chunks[k][:], wl_sb[:, KX + k, ns],
                                    start=False, stop=False,
                                )
                    # finish banks in ACT order for chunks 2,3
                    for n in BANK_ORDER:
                        ns = slice(n * 512, (n + 1) * 512)
                        for k in (2, 3):
                            nc.tensor.matmul(
                                zb[n][:], hT_chunks[k][:], wl_sb[:, KX + k, ns],
                                start=False, stop=(k == KH - 1),
                            )

                gates = sp.tile([128, G4], bf16, tag="gates")
                for n in BANK_ORDER:
                    nc.scalar.activation(
                        gates[:, n * 512:(n + 1) * 512], zb[n][:], GATE_FUNC[n]
                    )

                # elementwise tail, two H-halves; c*f on gpsimd to keep the
                # Vector queue short (i*j, c add, h mul)
                t2 = sp.tile([128, H], f32, tag="t2")
                if t == 0:
                    c_new = t2
                else:
                    t1 = sp.tile([128, H], f32, tag="t1")
                    c_new = sp.tile([128, H], f32, tag="c")
                last = t == t_steps - 1
                if not last:
                    tanc = sp.tile([128, H], bf16, tag="tanc")
                    h_new = [sp.tile([128, 256], bf16, tag=f"h{hf}",
                                     name=f"h{hf}")
                             for hf in (0, 1)]
                for hf in (0, 1):
                    sl = slice(hf * 256, (hf + 1) * 256)
                    gi = gates[:, hf * 256:hf * 256 + 256]
                    gf = gates[:, 512 + hf * 256:512 + hf * 256 + 256]
                    go = gates[:, 1024 + hf * 256:1024 + hf * 256 + 256]
                    gj = gates[:, 1536 + hf * 256:1536 + hf * 256 + 256]
                    nc.vector.tensor_mul(t2[:, sl], gi, gj)
                    if t > 0:
                        nc.gpsimd.tensor_mul(t1[:, sl], c_prev[:, sl], gf)
                        nc.vector.tensor_add(c_new[:, sl], t1[:, sl], t2[:, sl])
                    if not last:
                        nc.scalar.activation(tanc[:, sl], c_new[:, sl], AF.Tanh)
                        nc.vector.tensor_mul(h_new[hf][:], tanc[:, sl], go)
                c_prev = c_new
                if not last:
                    h_pend = h_new

            # final cell state -> bf16, transposed into cT [p, k*128+b]
            cb = sp.tile([128, H], bf16, tag="cb")
            nc.vector.tensor_copy(cb[:], c_prev[:])
            cT_bf = sp.tile([128, H], bf16, tag="cT")
            for k in range(4):
                ks = slice(k * 128, (k + 1) * 128)
                trp = trpool.tile([128, 128], bf16, tag="tr")
                nc.tensor.transpose(trp[:], cb[:, ks], idr_sb[:])
                nc.vector.tensor_copy(cT_bf[:, ks], trp[:])

        # layer-1 partial for this core's direction slice of W1, then
        # AllReduce the partial pre-activations within each batch-half group
        with tc.tile_pool(name="mlppsum", bufs=1, space="PSUM") as mp, \
             tc.tile_pool(name="mtrpsum", bufs=3, space="PSUM") as mtr:
            aps = mp.tile([128, F], f32, tag="aps1")
            for half in (0, 1):
                ms = slice(half * 512, (half + 1) * 512)
                for kc in range(4):
                    nc.tensor.matmul(
                        aps[:, ms], cT_bf[:, kc * 128:(kc + 1) * 128],
                        w1_sb[:, kc, ms], start=(kc == 0), stop=False,
                    )
                nc.tensor.matmul(aps[:, ms], ones_sb[0:1, :], b1_sb[0:1, ms],
                                 start=False, stop=True)
            # mask the partial into this core's batch-half slab (zeros into
            # the other) so ONE 8-core AllReduce serves both halves — the
            # SPMD program is identical on every core, the mask input isn't
            part1 = sp.tile([128, F], bf16, tag="part1")
            nc.vector.tensor_copy(part1[:], aps[:])
            pm = sp.tile([128, 2, F], bf16, tag="pm")
            for s in (0, 1):
                nc.vector.tensor_scalar(
                    pm[:, s], part1[:], m01_sb[:, s:s + 1], None,
                    mybir.AluOpType.mult,
                )
                nc.sync.dma_start(crin.ap()[s], pm[:, s])
            nc.gpsimd.collective_compute(
                "AllReduce",
                mybir.AluOpType.add,
                replica_groups=[[0, 1, 2, 3, 4, 5, 6, 7]],
                ins=[crin.ap()[:]],
                outs=[crout.ap()[:]],
            )
            s01 = sp.tile([128, 2, F], bf16, tag="s01")
            for s in (0, 1):
                nc.sync.dma_start(s01[:, s], crout.ap()[s])
            sm = sp.tile([128, 2, F], bf16, tag="sm")
            for s in (0, 1):
                nc.vector.tensor_scalar(
                    sm[:, s], s01[:, s], m01_sb[:, s:s + 1], None,
                    mybir.AluOpType.mult,
                )
            s1 = sp.tile([128, F], bf16, tag="s1")
            nc.vector.tensor_add(s1[:], sm[:, 0], sm[:, 1])
            act_in = sp.tile([128, F], bf16, tag="a1")
            nc.scalar.activation(act_in[:], s1[:], AF.Tanh)

            # layers 2..3 wide-N: transpose a, then psum = aT @ W + b, tanh
            for li, (w_sb, b_sb) in enumerate([(w2_sb, b2_sb), (w3_sb, b3_sb)]):
                aT = sp.tile([128, F], bf16, tag=f"aT{li}")
                for m in range(8):
                    ms = slice(m * 128, (m + 1) * 128)
                    trp = mtr.tile([128, 128], bf16, tag="mtr")
                    nc.tensor.transpose(trp[:], act_in[:, ms], idr_sb[:])
                    if m % 2 == 0:
                        nc.scalar.copy(aT[:, ms], trp[:])
                    else:
                        nc.vector.tensor_copy(aT[:, ms], trp[:])
                aps2 = mp.tile([128, F], f32, tag="apsL", name="aps2")
                for half in (0, 1):
                    ms = slice(half * 512, (half + 1) * 512)
                    for kc in range(8):
                        nc.tensor.matmul(
                            aps2[:, ms], aT[:, kc * 128:(kc + 1) * 128],
                            w_sb[:, kc, ms], start=(kc == 0), stop=False,
                        )
                    nc.tensor.matmul(aps2[:, ms], ones_sb[0:1, :], b_sb[0:1, ms],
                                     start=False, stop=True)
                nxt = sp.tile([128, F], bf16, tag=f"a{li + 2}")
                nc.scalar.activation(nxt[:], aps2[:], AF.Tanh)
                act_in = nxt

            # layer 4 in [3, batch] layout: stationary = W4 chunks
            a3T = sp.tile([128, F], bf16, tag="a3T")
            for m in range(8):
                ms = slice(m * 128, (m + 1) * 128)
                trp = mtr.tile([128, 128], bf16, tag="mtr")
                nc.tensor.transpose(trp[:], act_in[:, ms], idr_sb[:])
                if m % 2 == 0:
                    nc.scalar.copy(a3T[:, ms], trp[:])
                else:
                    nc.vector.tensor_copy(a3T[:, ms], trp[:])
            l4 = mp.tile([3, 128], f32, tag="l4")
            for kc in range(8):
                nc.tensor.matmul(
                    l4[:], w4_sb[:, kc, :], a3T[:, kc * 128:(kc + 1) * 128],
                    start=(kc == 0), stop=False,
                )
            nc.tensor.matmul(l4[:], b4_sb[0:1, :], ones_sb[0:1, :],
                             start=False, stop=True)
            lg = sp.tile([3, 128], f32, tag="lg")
            nc.scalar.copy(lg[:], l4[:])
            nc.sync.dma_start(out_d[:], lg[:])

    return nc


def _pack_core_inputs(core, inputs, t_steps=T):
    """Build the per-core in_map (numpy only)."""
    bf16 = ml_dtypes.bfloat16
    lstm = core % 4
    half = core // 4
    rows = slice(half * BC, (half + 1) * BC)

    if lstm < 2:
        x = np.asarray(inputs["premises"])[rows]
        W = np.asarray(inputs["W_fw_p"] if lstm == 0 else inputs["W_bw_p"])
        b = np.asarray(inputs["b_fw_p"] if lstm == 0 else inputs["b_bw_p"])
    else:
        x = np.asarray(inputs["hypotheses"])[rows]
        W = np.asarray(inputs["W_fw_h"] if lstm == 2 else inputs["W_bw_h"])
        b = np.asarray(inputs["b_fw_h"] if lstm == 2 else inputs["b_bw_h"])
    x = x[:, :t_steps]
    if lstm % 2 == 1:
        x = x[:, ::-1, :]

    # gate reorder [i, f, o, j]; fold forget_bias=1.0 into b
    perm = np.concatenate([
        np.arange(0, H), np.arange(2 * H, 3 * H),
        np.arange(3 * H, 4 * H), np.arange(H, 2 * H),
    ])
    Wp = W[:, perm].astype(np.float32)
    bp = b[perm].astype(np.float32).copy()
    bp[H:2 * H] += 1.0  # forget gate slice in new layout

    xa = np.zeros((BC, t_steps, EP), np.float32)
    xa[:, :, :E] = x
    xa[:, :, E] = 1.0
    xt = np.ascontiguousarray(
        xa.reshape(BC, t_steps, KX, 128).transpose(1, 3, 2, 0)
    ).reshape(t_steps, 128, KX * 128)

    wl = np.zeros((128, KX + KH, 4 * H), np.float32)
    W_aug_x = np.zeros((EP, 4 * H), np.float32)
    W_aug_x[:E] = Wp[:E]
    W_aug_x[E] = bp
    for k in range(KX):
        wl[:, k, :] = W_aug_x[k * 128:(k + 1) * 128]
    for k in range(KH):
        wl[:, KX + k, :] = Wp[E + k * 128: E + (k + 1) * 128]

    W1 = np.asarray(inputs["W1"]).astype(np.float32)
    W2 = np.asarray(inputs["W2"]).astype(np.float32)
    W3 = np.asarray(inputs["W3"]).astype(np.float32)
    W4 = np.asarray(inputs["W4"]).astype(np.float32)
    # per-core W1 slice: rows for this core's direction in the rnn concat
    w1s = W1[512 * lstm:512 * (lstm + 1)].reshape(4, 128, F).transpose(1, 0, 2)
    w2 = W2.reshape(8, 128, F).transpose(1, 0, 2).astype(bf16)
    w3 = W3.reshape(8, 128, F).transpose(1, 0, 2).astype(bf16)
    w4 = W4.reshape(8, 128, 3).transpose(1, 0, 2).astype(bf16)

    return {
        "xt": xt.astype(bf16),
        "wl": wl.astype(bf16),
        "w1": np.ascontiguousarray(w1s).astype(bf16),
        "w2": np.ascontiguousarray(w2),
        "w3": np.ascontiguousarray(w3),
        "w4": np.ascontiguousarray(w4),
        # b1 scaled by 1/4: each of the 4 group cores adds it once into the
        # AllReduce sum
        "b1": (np.asarray(inputs["b1"]).reshape(1, F) * 0.25).astype(bf16),
        "b2": np.asarray(inputs["b2"]).reshape(1, F).astype(bf16),
        "b3": np.asarray(inputs["b3"]).reshape(1, F).astype(bf16),
        "b4": np.asarray(inputs["b4"]).reshape(1, 3).astype(bf16),
        "ones": np.ones((1, 128), bf16),
        "m01": np.tile(np.array([[1.0 - half, float(half)]], dtype=np.float32), (128, 1)),
        "identr": np.eye(128, dtype=bf16),
    }


def _install_ntff_shim():
    """This image's `antenv` lacks `axon_hooks`; provide it so
    run_bass_kernel_spmd(trace=True) can capture NTFF profiles."""
    import sys
    import types

    if "antenv.axon_hooks" in sys.modules:
        return
    mod = types.ModuleType("antenv.axon_hooks")
    state = {"hook": None}
    mod.set_axon_ntff_profile_hook = lambda h: state.__setitem__("hook", h)
    mod.get_axon_ntff_profile_hook = lambda: state["hook"]
    sys.modules["antenv.axon_hooks"] = mod
    try:
        from trn_agent_boot.trn_boot import _ntff_profile_via_ctypes

        mod.set_axon_ntff_profile_hook(
            _ntff_profile_via_ctypes("/opt/axon/libaxon_pjrt.so")
        )
    except Exception:
        pass


def _run(inputs, trace=False, t_steps=T, ldt="bf16"):
    if trace:
        _install_ntff_shim()
    from concourse.bass_utils import run_bass_kernel_spmd

    key = (t_steps,)
    if key not in _cache:
        _cache[key] = _build_nc(t_steps)
    nc = _cache[key]
    in_maps = [_pack_core_inputs(c, inputs, t_steps) for c in range(N_CORES)]
    res = run_bass_kernel_spmd(
        nc, in_maps, list(range(N_CORES)), trace=trace
    )
    out = np.zeros((B, 3), np.float32)
    out[0:BC] = res.results[0]["logitsT"].T
    out[BC:2 * BC] = res.results[4]["logitsT"].T
    return out, res


def kernel(**inputs) -> np.ndarray:
    out, _ = _run(inputs, trace=False)
    return out
